# revision 13
# baseline (speedup 1.0000x reference)
"""Trainium2 Bass kernel for nn_ActorCritic (GIN actor-critic forward pass).

Sharding: data-parallel over the batch dim B=16 -> 2 graphs per core on 8
NeuronCores. Weights replicated. No collectives.

Device math (per graph, feature-major GNN, fp16 matmuls / f32 accumulate):
  q  = h.T @ A.T       (lhsT = h node-major; rhs = adjT, uploaded pre-shuffled
                        into the exact SBUF layout so the DMA is contiguous)
  z1 = W1.T @ q        -> t = relu(z1 + b1)   (ACT, per-partition bias)
  z2 = t.T @ W2 + b2   (node-major: t-slices stationary, bias via PSUM-prefill
                        matmul) -> h = relu(z2) on DVE
  pool + candidate gather fused in one matmul: rhs = [gp | onehot(cand)].
  actor/critic heads fp16; device (scatter) branch fp32 end-to-end.
Host: blob-packs weights/constants (2 DMAs), softmax/sampling tail (gumbel
trick with the reference's jax keys).
"""
import numpy as np

B, N, J, D, H, HA, HC, IN = 16, 1024, 128, 16, 128, 64, 64, 3
NCORES = 8
BL = B // NCORES  # graphs per core

_cache = {}

# ---- f16 blob column layout ----
F16_COLS = {}
_c = 0
def _f16(name, cols):
    global _c
    F16_COLS[name] = (_c, cols)
    _c += cols
for _l in range(3):
    _f16(f"g{_l}W1", H)
for _l in range(3):
    _f16(f"g{_l}W2", H)
for _l in range(3):
    _f16(f"g{_l}b2rep", N)
_f16("aW1t", HA); _f16("aW1b", HA); _f16("aW2", 1)
_f16("cW1", HC); _f16("cW2", 1)
_f16("ones1", 128)
_f16("h0", BL * 8 * IN)
_f16("cr0", J); _f16("cr1", J)
_f16("gp0", 8); _f16("gp1", 8)
C16 = _c

# ---- f32 blob column layout ----
F32_COLS = {}
_c = 0
def _f32(name, cols):
    global _c
    F32_COLS[name] = (_c, cols)
    _c += cols
_f32("sft", BL * 8 * IN)
_f32("fm0", D); _f32("fm1", D)
_f32("iota16", D)
_f32("iota8", 8)
for _l in range(3):
    _f32(f"g{_l}b1", 1)
_f32("ab1", 1); _f32("ab2", 1); _f32("cb1", 1); _f32("cb2", 1)
_f32("aplW1e", 8 * HA); _f32("cplW1e", 8 * HA)
_f32("aplW1hw", HA); _f32("cplW1hw", HA)
_f32("aplb1", 1); _f32("aplW2", 1); _f32("aplb2", 1)
_f32("cplb1", 1); _f32("cplW2", 1); _f32("cplb2", 1)
C32 = _c


def _build_nc():
    import concourse.mybir as mybir
    from concourse import bacc
    from concourse.tile import TileContext

    f32 = mybir.dt.float32
    f16 = mybir.dt.float16
    AF = mybir.ActivationFunctionType
    ALU = mybir.AluOpType

    nc = bacc.Bacc("TRN2", target_bir_lowering=False, debug=False)

    wf16_e = nc.declare_dram_parameter("wf16", [128, C16], f16, isOutput=False)
    wf32_e = nc.declare_dram_parameter("wf32", [128, C32], f32, isOutput=False)
    adjT_e = nc.declare_dram_parameter("adjT", [BL, 128, 8192], f16, isOutput=False)
    out_e = nc.declare_dram_parameter("outs", [BL, 161], f32, isOutput=True)

    with TileContext(nc) as tc:
        with tc.tile_pool(name="sb", bufs=1) as sb, \
             tc.tile_pool(name="ps", bufs=1, space="PSUM") as ps:

            warm = sb.tile([128, 512], f16, tag="warm")
            nc.vector.memset(warm[:], 0.0)
            # two short warm bursts: kernel start + just before the first
            # A-matmuls land, so HAM is at K=8/8 when real work begins
            for burst, t_at in ((0, 0.0), (1, 0.004), (2, 0.0075)):
                wps = ps.tile([128, 512], f32, tag="qc", bufs=3,
                              name=f"warmps{burst}")
                with tc.tile_wait_until(t_at):
                    for wi in range(8):
                        nc.tensor.matmul(wps[:], warm[:, 0:128], warm[:],
                                         start=(wi == 0), stop=(wi == 7))

            wf16 = sb.tile([128, C16], f16, tag="wf16")
            nc.sync.dma_start(wf16[:], wf16_e[:])
            wf32 = sb.tile([128, C32], f32, tag="wf32")
            nc.sync.dma_start(wf32[:], wf32_e[:])

            def W16(name, rows=128):
                c0, cn = F16_COLS[name]
                return wf16[:rows, c0:c0 + cn]

            def W32(name, rows=128):
                c0, cn = F32_COLS[name]
                return wf32[:rows, c0:c0 + cn]

            At = []
            for g in range(BL):
                t_ = sb.tile([128, 8192], f16, tag=f"at{g}", name=f"at{g}")
                for qt in range(4):
                    nc.sync.dma_start(t_[:, qt * 2048:(qt + 1) * 2048],
                                      adjT_e[g, :, qt * 2048:(qt + 1) * 2048])
                At.append(t_)

            out_sb = []
            for g in range(BL):
                o_ = sb.tile([1, 161], f32, tag=f"out{g}", name=f"out{g}")
                out_sb.append(o_)

            # ---- prebuild gather rhs tiles (only need the blobs) ----
            Rt = {}
            for g in range(BL):
                for jb in range(8):
                    R = sb.tile([128, 1 + J], f16, tag=f"R{g}_{jb}",
                                name=f"R{g}_{jb}")
                    nc.vector.tensor_copy(R[:, 0:1], W16(f"gp{g}")[:, jb:jb + 1])
                    nc.vector.tensor_scalar(
                        R[:, 1:1 + J], W16(f"cr{g}"), W32("iota8")[:, jb:jb + 1],
                        None, ALU.is_equal)
                    Rt[g, jb] = R

            # ---- device (scatter) branch, fp32 — runs under the adjT DMAs
            for g in range(BL):
                Mt = []
                for jb in range(8):
                    m_ = sb.tile([128, D], f32, tag=f"mt{g}_{jb}", name=f"mt{g}_{jb}")
                    gib = g * 8 + jb
                    base = F32_COLS["sft"][0] + gib * IN
                    nc.vector.tensor_scalar(
                        m_[:], W32("iota16"), wf32[:, base + 2:base + 3],
                        wf32[:, base:base + 1], ALU.is_equal, ALU.mult)
                    Mt.append(m_)
                for p, osl in (("apl", slice(129, 145)), ("cpl", slice(145, 161))):
                    y1 = ps.tile([HA, D], f32, tag="z1c", bufs=2, name=f"y1{g}{p}")
                    w1e = W32(f"{p}W1e")
                    for jb in range(8):
                        nc.tensor.matmul(y1[:], w1e[:, jb * HA:(jb + 1) * HA],
                                         Mt[jb][:], start=(jb == 0), stop=False)
                    nc.tensor.matmul(y1[:], W32(f"{p}W1hw", 2),
                                     W32(f"fm{g}", 2), start=False, stop=True)
                    tpl = sb.tile([HA, D], f32, tag=f"tpl{g}_{p}", name=f"tpl{g}_{p}")
                    nc.scalar.activation(tpl[:], y1[:], AF.Tanh,
                                         bias=W32(f"{p}b1", HA))
                    yp = ps.tile([1, D], f32, tag="z2c", bufs=3, name=f"yp{g}{p}")
                    nc.tensor.matmul(yp[:], W32(f"{p}W2", HA), tpl[:],
                                     start=True, stop=True)
                    nc.scalar.activation(out_sb[g][:, osl], yp[:], AF.Identity,
                                         bias=W32(f"{p}b2", 1))

            # ---- GNN, chunk-pipelined, graphs interleaved ----
            def At_rhs(g, jb, lo, width):
                base = (jb // 4) * 4096 + (jb % 4) * N
                return At[g][:, base + lo: base + lo + width]

            h_nm = {g: None for g in range(BL)}
            for l in range(3):
                din = IN if l == 0 else H
                qb, t_fm, h_new = {}, {}, {}
                for g in range(BL):
                    qb[g] = sb.tile([din, N], f16, tag="qb", bufs=3,
                                    name=f"qb{g}_{l}")
                    t_fm[g] = sb.tile([128, N], f16, tag="t", bufs=3,
                                      name=f"t{g}_{l}")
                    h_new[g] = sb.tile([128, N], f16, tag="h", bufs=4,
                                       name=f"h{g}_{l}")
                for c in range(2):
                    for g in range(BL):
                        cs = slice(c * 512, (c + 1) * 512)
                        q = ps.tile([128, 512], f32, tag="qc", bufs=3,
                                    name=f"q{g}_{l}_{c}")
                        for jb in range(8):
                            if l == 0:
                                b0 = F16_COLS["h0"][0] + (g * 8 + jb) * IN
                                lhsT = wf16[:, b0:b0 + IN]
                            else:
                                lhsT = h_nm[g][:, jb * H:(jb + 1) * H]
                            nc.tensor.matmul(q[:din, :], lhsT,
                                             At_rhs(g, jb, c * 512, 512),
                                             start=(jb == 0), stop=(jb == 7))
                        nc.vector.tensor_copy(qb[g][:, cs], q[:din, :])
                        z1 = ps.tile([128, 512], f32, tag="z1c", bufs=2,
                                     name=f"z1{g}_{l}_{c}")
                        nc.tensor.matmul(z1[:], W16(f"g{l}W1", din),
                                         qb[g][:, cs], start=True, stop=True)
                        nc.scalar.activation(t_fm[g][:, cs], z1[:], AF.Relu,
                                             bias=W32(f"g{l}b1", H))
                        z2 = ps.tile([128, 512], f32, tag="z2c", bufs=3,
                                     name=f"z2{g}_{l}_{c}")
                        nc.tensor.matmul(z2[:], W16("ones1", 1),
                                         W16(f"g{l}b2rep", 1)[:, cs],
                                         start=True, stop=False)
                        for k in range(4):
                            ib = 4 * c + k
                            nc.tensor.matmul(
                                z2[:, k * H:(k + 1) * H],
                                t_fm[g][:, ib * H:(ib + 1) * H],
                                W16(f"g{l}W2", H), start=False, stop=True)
                        nc.vector.tensor_scalar(h_new[g][:, cs], z2[:], 0.0,
                                                None, ALU.max)
                for g in range(BL):
                    h_nm[g] = h_new[g]

            for g in range(BL):
                # ---- pool + candidate gather ----
                P = ps.tile([128, 1 + J], f32, tag="z1c", bufs=2, name=f"P{g}")
                for jb in range(8):
                    nc.tensor.matmul(P[:], h_nm[g][:, jb * H:(jb + 1) * H],
                                     Rt[g, jb][:], start=(jb == 0),
                                     stop=(jb == 7))
                Pb = sb.tile([128, 1 + J], f16, tag=f"Pb{g}")
                nc.vector.tensor_copy(Pb[:], P[:])

                # ---- actor head ----
                zcf = ps.tile([HA, J], f32, tag="z2c", bufs=3, name=f"zcf{g}")
                nc.tensor.matmul(zcf[:], W16("aW1t"), Pb[:, 1:1 + J],
                                 start=True, stop=True)
                zhp = ps.tile([HA, 1], f32, tag="qc", bufs=3, name=f"zhp{g}")
                nc.tensor.matmul(zhp[:], W16("aW1b"), Pb[:, 0:1],
                                 start=True, stop=True)
                bias_a = sb.tile([HA, 1], f32, tag=f"biasa{g}")
                nc.vector.tensor_add(bias_a[:], zhp[:], W32("ab1", HA))
                ta = sb.tile([HA, J], f16, tag=f"ta{g}")
                nc.scalar.activation(ta[:], zcf[:], AF.Tanh, bias=bias_a[:])
                scp = ps.tile([1, J], f32, tag="qc", bufs=3, name=f"scp{g}")
                nc.tensor.matmul(scp[:], W16("aW2", HA), ta[:],
                                 start=True, stop=True)
                nc.scalar.activation(out_sb[g][:, 0:J], scp[:], AF.Identity,
                                     bias=W32("ab2", 1))

                # ---- critic head ----
                zv = ps.tile([HC, 1], f32, tag="qc", bufs=3, name=f"zv{g}")
                nc.tensor.matmul(zv[:], W16("cW1"), Pb[:, 0:1],
                                 start=True, stop=True)
                tv = sb.tile([HC, 1], f16, tag=f"tv{g}")
                nc.scalar.activation(tv[:], zv[:], AF.Tanh, bias=W32("cb1", HC))
                vv = ps.tile([1, 1], f32, tag="qc", bufs=3, name=f"vv{g}")
                nc.tensor.matmul(vv[:], W16("cW2", HC), tv[:],
                                 start=True, stop=True)
                nc.scalar.activation(out_sb[g][:, J:J + 1], vv[:], AF.Identity,
                                     bias=W32("cb2", 1))
                nc.sync.dma_start(out_e[g:g + 1, :], out_sb[g][:])

    nc.compile()
    return nc


def _flatten_params(params):
    out = {}
    for l, layer in enumerate(params["gnn"]):
        for k, v in layer.items():
            out[f"gnn{l}_{k}"] = v
    for head in ("actor", "critic", "actorPL", "criticPL"):
        for k, v in params[head].items():
            out[f"{head}_{k}"] = v
    return out


def _prep_in_maps(state_ft, state_fm, candidate, mask, adj, graph_pool, params):
    f16 = np.float16
    state_ft = np.asarray(state_ft, np.float32)
    state_fm = np.asarray(state_fm, np.float32)
    candidate_np = np.asarray(candidate)
    adj = np.asarray(adj, np.float32)
    graph_pool = np.asarray(graph_pool, np.float32)
    P = {k: np.asarray(v, np.float32) for k, v in _flatten_params(params).items()}

    w16 = np.zeros((128, C16), f16)
    w32 = np.zeros((128, C32), np.float32)

    def put16(name, arr):
        c0, cn = F16_COLS[name]
        w16[:arr.shape[0], c0:c0 + arr.shape[1]] = arr.astype(f16)

    def put32(name, arr):
        c0, cn = F32_COLS[name]
        w32[:arr.shape[0], c0:c0 + arr.shape[1]] = arr

    for l in range(3):
        put16(f"g{l}W1", P[f"gnn{l}_W1"])
        put16(f"g{l}W2", P[f"gnn{l}_W2"])
        put16(f"g{l}b2rep", np.tile(P[f"gnn{l}_b2"], 8)[None, :])
        put32(f"g{l}b1", P[f"gnn{l}_b1"][:, None])
    put16("aW1t", P["actor_W1"][:H]); put16("aW1b", P["actor_W1"][H:])
    put16("aW2", P["actor_W2"]); put16("cW1", P["critic_W1"])
    put16("cW2", P["critic_W2"])
    put16("ones1", np.ones((1, 128), np.float32))
    put32("iota8", np.arange(8, dtype=np.float32)[None, :] * 128
          + np.arange(128, dtype=np.float32)[:, None])
    put32("ab1", P["actor_b1"][:, None]); put32("ab2", P["actor_b2"][:, None])
    put32("cb1", P["critic_b1"][:, None]); put32("cb2", P["critic_b2"][:, None])
    put32("iota16", np.broadcast_to(
        np.arange(D, dtype=np.float32), (128, D)).copy())
    for p, nm in (("apl", "actorPL"), ("cpl", "criticPL")):
        w1e = P[f"{nm}_W1"][2::2]                      # (N, HA)
        put32(f"{p}W1e", np.ascontiguousarray(
            w1e.reshape(8, 128, HA).transpose(1, 0, 2).reshape(128, 8 * HA)))
        put32(f"{p}W1hw", P[f"{nm}_W1"][:2])
        put32(f"{p}b1", P[f"{nm}_b1"][:, None])
        put32(f"{p}W2", P[f"{nm}_W2"])
        put32(f"{p}b2", P[f"{nm}_b2"][:, None])

    sft3 = state_ft.reshape(B, N, IN)
    fm3 = state_fm.reshape(B, D, 2)
    in_maps = []
    for c in range(NCORES):
        sl = slice(c * BL, (c + 1) * BL)
        wc16 = w16.copy()
        wc32 = w32.copy()
        # sft layout: col = (g*8 + b)*3 + ch;  [128, 48]
        s = np.ascontiguousarray(
            sft3[sl].reshape(BL * 8, 128, IN).transpose(1, 0, 2)
            .reshape(128, BL * 8 * IN))
        c0 = F32_COLS["sft"][0]
        wc32[:, c0:c0 + BL * 8 * IN] = s
        c0 = F16_COLS["h0"][0]
        wc16[:, c0:c0 + BL * 8 * IN] = s.astype(f16)
        for g in range(BL):
            gg = c * BL + g
            c0, _ = F16_COLS[f"cr{g}"]
            wc16[:, c0:c0 + J] = candidate_np[gg].astype(f16)[None, :]
            c0, _ = F16_COLS[f"gp{g}"]
            wc16[:, c0:c0 + 8] = graph_pool[gg].reshape(8, 128).T.astype(f16)
            c0, _ = F32_COLS[f"fm{g}"]
            wc32[:2, c0:c0 + D] = fm3[gg].T
        # adjT pre-shuffled to the exact SBUF layout: row p holds, for each
        # half hf and block b, AT[512*hf + 128*b + p, :]  (AT = adj.T)
        a = adj[sl].transpose(0, 2, 1).reshape(BL, 2, 4, 128, N)
        a = a.transpose(0, 3, 1, 2, 4).reshape(BL, 128, 8192)
        in_maps.append({
            "wf16": wc16, "wf32": wc32,
            "adjT": np.ascontiguousarray(a).astype(f16),
        })
    return in_maps


def _install_ntff_hook():
    import sys, types
    if "antenv.axon_hooks" in sys.modules:
        return
    try:
        import antenv
        mod = types.ModuleType("antenv.axon_hooks")
        mod._hook = None
        mod.set_axon_ntff_profile_hook = lambda h: setattr(mod, "_hook", h)
        mod.get_axon_ntff_profile_hook = lambda: mod._hook
        sys.modules["antenv.axon_hooks"] = mod
        antenv.axon_hooks = mod
        from trn_agent_boot.trn_boot import _ntff_profile_via_ctypes
        mod.set_axon_ntff_profile_hook(
            _ntff_profile_via_ctypes("/opt/axon/libaxon_pjrt.so"))
    except Exception:
        pass


def _gumbel_noise():
    if "gumbel" not in _cache:
        import jax
        k1, k2 = jax.random.split(jax.random.key(42))
        g1 = np.asarray(jax.random.gumbel(k1, (B, J), np.float32))
        g2 = np.asarray(jax.random.gumbel(k2, (B, D), np.float32))
        _cache["gumbel"] = (g1, g2)
    return _cache["gumbel"]


def _run_device(in_maps, trace=False):
    from concourse.bass_utils import run_bass_kernel_spmd
    if "nc" not in _cache:
        _cache["nc"] = _build_nc()
    if trace:
        _install_ntff_hook()
    res = run_bass_kernel_spmd(
        _cache["nc"], in_maps, core_ids=list(range(NCORES)), trace=trace)
    O = np.concatenate([r["outs"] for r in res.results], 0)   # (B, 161)
    S = O[:, 0:J]
    V = O[:, J:J + 1]
    DS = O[:, J + 1:J + 1 + D]
    CR = O[:, J + 1 + D:J + 1 + 2 * D]
    return (S, V, DS, CR), res


def _host_tail(S, V, DS, CR, candidate, mask):
    mask = np.asarray(mask, bool)
    cand = np.asarray(candidate)
    g1, g2 = _gumbel_noise()
    logits = np.where(mask, -np.inf, S)
    zmax = np.max(logits, axis=1, keepdims=True)
    e = np.exp(logits - zmax)
    e[~np.isfinite(logits)] = 0.0
    esum = e.sum(1, keepdims=True)
    pi = (e / esum)[..., None].astype(np.float32)
    task_ix = np.argmax(logits + g1, axis=1).astype(np.int32)
    lse = zmax[:, 0] + np.log(esum[:, 0])
    logp = (np.take_along_axis(logits, task_ix[:, None].astype(np.int64), 1)[:, 0]
            - lse).astype(np.float32)
    sel = np.take_along_axis(cand, task_ix[:, None].astype(np.int64), 1)[:, 0]
    sel = sel.astype(np.int32)
    dmax = DS.max(1, keepdims=True)
    ed = np.exp(DS - dmax)
    edsum = ed.sum(1, keepdims=True)
    mhi = (ed / edsum)[..., None].astype(np.float32)
    device_ID = np.argmax(DS + g2, axis=1).astype(np.int32)
    mh_logp = (np.take_along_axis(DS, device_ID[:, None].astype(np.int64), 1)[:, 0]
               - (dmax[:, 0] + np.log(edsum[:, 0]))).astype(np.float32)
    vm = CR.min(1).astype(np.float32)
    v = V.astype(np.float32).reshape(B, 1)
    return (sel, task_ix, pi, v, logp, device_ID, mhi, vm, mh_logp)


def kernel(state_ft, state_fm, candidate, mask, adj, graph_pool, params):
    in_maps = _prep_in_maps(state_ft, state_fm, candidate, mask, adj,
                            graph_pool, params)
    (S, V, DS, CR), _ = _run_device(in_maps)
    return _host_tail(S, V, DS, CR, candidate, mask)


# revision 14
# speedup vs baseline: 1.1630x; 1.1630x over previous
"""Trainium2 Bass kernel for nn_ActorCritic (GIN actor-critic forward pass).

Sharding: data-parallel over the batch dim B=16 -> 2 graphs per core on 8
NeuronCores. Weights replicated. No collectives.

Device math (per graph, feature-major GNN, fp16 matmuls / f32 accumulate):
  q  = h.T @ A.T       (lhsT = h node-major; rhs = adjT, uploaded pre-shuffled
                        into the exact SBUF layout so the DMA is contiguous)
  z1 = W1.T @ q        -> t = relu(z1 + b1)   (ACT, per-partition bias)
  z2 = t.T @ W2 + b2   (node-major: t-slices stationary, bias via PSUM-prefill
                        matmul) -> h = relu(z2) on DVE
  pool + candidate gather fused in one matmul: rhs = [gp | onehot(cand)].
  actor/critic heads fp16; device (scatter) branch fp32 end-to-end.
Host: blob-packs weights/constants (2 DMAs), softmax/sampling tail (gumbel
trick with the reference's jax keys).
"""
import numpy as np

B, N, J, D, H, HA, HC, IN = 16, 1024, 128, 16, 128, 64, 64, 3
NCORES = 8
BL = B // NCORES  # graphs per core

_cache = {}

# ---- f16 blob column layout ----
F16_COLS = {}
_c = 0
def _f16(name, cols):
    global _c
    F16_COLS[name] = (_c, cols)
    _c += cols
for _l in range(3):
    _f16(f"g{_l}W1", H)
for _l in range(3):
    _f16(f"g{_l}W2", H)
for _l in range(3):
    _f16(f"g{_l}b2rep", N)
_f16("aW1t", HA); _f16("aW1b", HA); _f16("aW2", 1)
_f16("cW1", HC); _f16("cW2", 1)
_f16("ones1", 128)
_f16("h0", BL * 8 * IN)
_f16("cr0", J); _f16("cr1", J)
_f16("gp0", 8); _f16("gp1", 8)
C16 = _c

# ---- f32 blob column layout ----
F32_COLS = {}
_c = 0
def _f32(name, cols):
    global _c
    F32_COLS[name] = (_c, cols)
    _c += cols
_f32("sft", BL * 8 * IN)
_f32("fm0", D); _f32("fm1", D)
_f32("iota16", D)
_f32("iota8", 8)
for _l in range(3):
    _f32(f"g{_l}b1", 1)
_f32("ab1", 1); _f32("ab2", 1); _f32("cb1", 1); _f32("cb2", 1)
_f32("aplW1e", 8 * HA); _f32("cplW1e", 8 * HA)
_f32("aplW1hw", HA); _f32("cplW1hw", HA)
_f32("aplb1", 1); _f32("aplW2", 1); _f32("aplb2", 1)
_f32("cplb1", 1); _f32("cplW2", 1); _f32("cplb2", 1)
C32 = _c


def _build_nc():
    import concourse.mybir as mybir
    from concourse import bacc
    from concourse.tile import TileContext

    f32 = mybir.dt.float32
    f16 = mybir.dt.float16
    AF = mybir.ActivationFunctionType
    ALU = mybir.AluOpType

    nc = bacc.Bacc("TRN2", target_bir_lowering=False, debug=False)

    wf16_e = nc.declare_dram_parameter("wf16", [128, C16], f16, isOutput=False)
    wf32_e = nc.declare_dram_parameter("wf32", [128, C32], f32, isOutput=False)
    adjT_e = nc.declare_dram_parameter("adjT", [BL, 128, 8192], f16, isOutput=False)
    out_e = nc.declare_dram_parameter("outs", [BL, 161], f32, isOutput=True)

    with TileContext(nc) as tc:
        with tc.tile_pool(name="sb", bufs=1) as sb, \
             tc.tile_pool(name="ps", bufs=1, space="PSUM") as ps:

            wf16 = sb.tile([128, C16], f16, tag="wf16")
            nc.sync.dma_start(wf16[:], wf16_e[:])
            wf32 = sb.tile([128, C32], f32, tag="wf32")
            nc.sync.dma_start(wf32[:], wf32_e[:])

            def W16(name, rows=128):
                c0, cn = F16_COLS[name]
                return wf16[:rows, c0:c0 + cn]

            def W32(name, rows=128):
                c0, cn = F32_COLS[name]
                return wf32[:rows, c0:c0 + cn]

            At = []
            for g in range(BL):
                t_ = sb.tile([128, 8192], f16, tag=f"at{g}", name=f"at{g}")
                for qt in range(4):
                    nc.sync.dma_start(t_[:, qt * 2048:(qt + 1) * 2048],
                                      adjT_e[g, :, qt * 2048:(qt + 1) * 2048])
                At.append(t_)

            out_sb = []
            for g in range(BL):
                o_ = sb.tile([1, 161], f32, tag=f"out{g}", name=f"out{g}")
                out_sb.append(o_)

            # ---- prebuild gather rhs tiles (only need the blobs) ----
            Rt = {}
            for g in range(BL):
                for jb in range(8):
                    R = sb.tile([128, 1 + J], f16, tag=f"R{g}_{jb}",
                                name=f"R{g}_{jb}")
                    nc.vector.tensor_copy(R[:, 0:1], W16(f"gp{g}")[:, jb:jb + 1])
                    nc.vector.tensor_scalar(
                        R[:, 1:1 + J], W16(f"cr{g}"), W32("iota8")[:, jb:jb + 1],
                        None, ALU.is_equal)
                    Rt[g, jb] = R

            # ---- device (scatter) branch, fp32 — runs under the adjT DMAs
            for g in range(BL):
                Mt = []
                for jb in range(8):
                    m_ = sb.tile([128, D], f32, tag=f"mt{g}_{jb}", name=f"mt{g}_{jb}")
                    gib = g * 8 + jb
                    base = F32_COLS["sft"][0] + gib * IN
                    nc.vector.tensor_scalar(
                        m_[:], W32("iota16"), wf32[:, base + 2:base + 3],
                        wf32[:, base:base + 1], ALU.is_equal, ALU.mult)
                    Mt.append(m_)
                for p, osl in (("apl", slice(129, 145)), ("cpl", slice(145, 161))):
                    y1 = ps.tile([HA, D], f32, tag="z1c", bufs=2, name=f"y1{g}{p}")
                    w1e = W32(f"{p}W1e")
                    for jb in range(8):
                        nc.tensor.matmul(y1[:], w1e[:, jb * HA:(jb + 1) * HA],
                                         Mt[jb][:], start=(jb == 0), stop=False)
                    nc.tensor.matmul(y1[:], W32(f"{p}W1hw", 2),
                                     W32(f"fm{g}", 2), start=False, stop=True)
                    tpl = sb.tile([HA, D], f32, tag=f"tpl{g}_{p}", name=f"tpl{g}_{p}")
                    nc.scalar.activation(tpl[:], y1[:], AF.Tanh,
                                         bias=W32(f"{p}b1", HA))
                    yp = ps.tile([1, D], f32, tag="z2c", bufs=3, name=f"yp{g}{p}")
                    nc.tensor.matmul(yp[:], W32(f"{p}W2", HA), tpl[:],
                                     start=True, stop=True)
                    nc.scalar.activation(out_sb[g][:, osl], yp[:], AF.Identity,
                                         bias=W32(f"{p}b2", 1))

            # ---- GNN, chunk-pipelined, graphs interleaved ----
            def At_rhs(g, jb, lo, width):
                base = (jb // 4) * 4096 + (jb % 4) * N
                return At[g][:, base + lo: base + lo + width]

            h_nm = {g: None for g in range(BL)}
            for l in range(3):
                din = IN if l == 0 else H
                qb, t_fm, h_new = {}, {}, {}
                for g in range(BL):
                    qb[g] = sb.tile([din, N], f16, tag="qb", bufs=3,
                                    name=f"qb{g}_{l}")
                    t_fm[g] = sb.tile([128, N], f16, tag="t", bufs=3,
                                      name=f"t{g}_{l}")
                    h_new[g] = sb.tile([128, N], f16, tag="h", bufs=4,
                                       name=f"h{g}_{l}")
                for c in range(2):
                    for g in range(BL):
                        cs = slice(c * 512, (c + 1) * 512)
                        q = ps.tile([128, 512], f32, tag="qc", bufs=3,
                                    name=f"q{g}_{l}_{c}")
                        for jb in range(8):
                            if l == 0:
                                b0 = F16_COLS["h0"][0] + (g * 8 + jb) * IN
                                lhsT = wf16[:, b0:b0 + IN]
                            else:
                                lhsT = h_nm[g][:, jb * H:(jb + 1) * H]
                            nc.tensor.matmul(q[:din, :], lhsT,
                                             At_rhs(g, jb, c * 512, 512),
                                             start=(jb == 0), stop=(jb == 7))
                        nc.vector.tensor_copy(qb[g][:, cs], q[:din, :])
                        z1 = ps.tile([128, 512], f32, tag="z1c", bufs=2,
                                     name=f"z1{g}_{l}_{c}")
                        nc.tensor.matmul(z1[:], W16(f"g{l}W1", din),
                                         qb[g][:, cs], start=True, stop=True)
                        nc.scalar.activation(t_fm[g][:, cs], z1[:], AF.Relu,
                                             bias=W32(f"g{l}b1", H))
                        z2 = ps.tile([128, 512], f32, tag="z2c", bufs=3,
                                     name=f"z2{g}_{l}_{c}")
                        nc.tensor.matmul(z2[:], W16("ones1", 1),
                                         W16(f"g{l}b2rep", 1)[:, cs],
                                         start=True, stop=False)
                        for k in range(4):
                            ib = 4 * c + k
                            nc.tensor.matmul(
                                z2[:, k * H:(k + 1) * H],
                                t_fm[g][:, ib * H:(ib + 1) * H],
                                W16(f"g{l}W2", H), start=False, stop=True)
                        nc.vector.tensor_scalar(h_new[g][:, cs], z2[:], 0.0,
                                                None, ALU.max)
                for g in range(BL):
                    h_nm[g] = h_new[g]

            for g in range(BL):
                # ---- pool + candidate gather ----
                P = ps.tile([128, 1 + J], f32, tag="z1c", bufs=2, name=f"P{g}")
                for jb in range(8):
                    nc.tensor.matmul(P[:], h_nm[g][:, jb * H:(jb + 1) * H],
                                     Rt[g, jb][:], start=(jb == 0),
                                     stop=(jb == 7))
                Pb = sb.tile([128, 1 + J], f16, tag=f"Pb{g}")
                nc.vector.tensor_copy(Pb[:], P[:])

                # ---- actor head ----
                zcf = ps.tile([HA, J], f32, tag="z2c", bufs=3, name=f"zcf{g}")
                nc.tensor.matmul(zcf[:], W16("aW1t"), Pb[:, 1:1 + J],
                                 start=True, stop=True)
                zhp = ps.tile([HA, 1], f32, tag="qc", bufs=3, name=f"zhp{g}")
                nc.tensor.matmul(zhp[:], W16("aW1b"), Pb[:, 0:1],
                                 start=True, stop=True)
                bias_a = sb.tile([HA, 1], f32, tag=f"biasa{g}")
                nc.vector.tensor_add(bias_a[:], zhp[:], W32("ab1", HA))
                ta = sb.tile([HA, J], f16, tag=f"ta{g}")
                nc.scalar.activation(ta[:], zcf[:], AF.Tanh, bias=bias_a[:])
                scp = ps.tile([1, J], f32, tag="qc", bufs=3, name=f"scp{g}")
                nc.tensor.matmul(scp[:], W16("aW2", HA), ta[:],
                                 start=True, stop=True)
                nc.scalar.activation(out_sb[g][:, 0:J], scp[:], AF.Identity,
                                     bias=W32("ab2", 1))

                # ---- critic head ----
                zv = ps.tile([HC, 1], f32, tag="qc", bufs=3, name=f"zv{g}")
                nc.tensor.matmul(zv[:], W16("cW1"), Pb[:, 0:1],
                                 start=True, stop=True)
                tv = sb.tile([HC, 1], f16, tag=f"tv{g}")
                nc.scalar.activation(tv[:], zv[:], AF.Tanh, bias=W32("cb1", HC))
                vv = ps.tile([1, 1], f32, tag="qc", bufs=3, name=f"vv{g}")
                nc.tensor.matmul(vv[:], W16("cW2", HC), tv[:],
                                 start=True, stop=True)
                nc.scalar.activation(out_sb[g][:, J:J + 1], vv[:], AF.Identity,
                                     bias=W32("cb2", 1))
                nc.sync.dma_start(out_e[g:g + 1, :], out_sb[g][:])

    nc.compile()
    return nc


def _flatten_params(params):
    out = {}
    for l, layer in enumerate(params["gnn"]):
        for k, v in layer.items():
            out[f"gnn{l}_{k}"] = v
    for head in ("actor", "critic", "actorPL", "criticPL"):
        for k, v in params[head].items():
            out[f"{head}_{k}"] = v
    return out


def _prep_in_maps(state_ft, state_fm, candidate, mask, adj, graph_pool, params):
    f16 = np.float16
    state_ft = np.asarray(state_ft, np.float32)
    state_fm = np.asarray(state_fm, np.float32)
    candidate_np = np.asarray(candidate)
    adj = np.asarray(adj, np.float32)
    graph_pool = np.asarray(graph_pool, np.float32)
    P = {k: np.asarray(v, np.float32) for k, v in _flatten_params(params).items()}

    w16 = np.zeros((128, C16), f16)
    w32 = np.zeros((128, C32), np.float32)

    def put16(name, arr):
        c0, cn = F16_COLS[name]
        w16[:arr.shape[0], c0:c0 + arr.shape[1]] = arr.astype(f16)

    def put32(name, arr):
        c0, cn = F32_COLS[name]
        w32[:arr.shape[0], c0:c0 + arr.shape[1]] = arr

    for l in range(3):
        put16(f"g{l}W1", P[f"gnn{l}_W1"])
        put16(f"g{l}W2", P[f"gnn{l}_W2"])
        put16(f"g{l}b2rep", np.tile(P[f"gnn{l}_b2"], 8)[None, :])
        put32(f"g{l}b1", P[f"gnn{l}_b1"][:, None])
    put16("aW1t", P["actor_W1"][:H]); put16("aW1b", P["actor_W1"][H:])
    put16("aW2", P["actor_W2"]); put16("cW1", P["critic_W1"])
    put16("cW2", P["critic_W2"])
    put16("ones1", np.ones((1, 128), np.float32))
    put32("iota8", np.arange(8, dtype=np.float32)[None, :] * 128
          + np.arange(128, dtype=np.float32)[:, None])
    put32("ab1", P["actor_b1"][:, None]); put32("ab2", P["actor_b2"][:, None])
    put32("cb1", P["critic_b1"][:, None]); put32("cb2", P["critic_b2"][:, None])
    put32("iota16", np.broadcast_to(
        np.arange(D, dtype=np.float32), (128, D)).copy())
    for p, nm in (("apl", "actorPL"), ("cpl", "criticPL")):
        w1e = P[f"{nm}_W1"][2::2]                      # (N, HA)
        put32(f"{p}W1e", np.ascontiguousarray(
            w1e.reshape(8, 128, HA).transpose(1, 0, 2).reshape(128, 8 * HA)))
        put32(f"{p}W1hw", P[f"{nm}_W1"][:2])
        put32(f"{p}b1", P[f"{nm}_b1"][:, None])
        put32(f"{p}W2", P[f"{nm}_W2"])
        put32(f"{p}b2", P[f"{nm}_b2"][:, None])

    sft3 = state_ft.reshape(B, N, IN)
    fm3 = state_fm.reshape(B, D, 2)
    in_maps = []
    for c in range(NCORES):
        sl = slice(c * BL, (c + 1) * BL)
        wc16 = w16.copy()
        wc32 = w32.copy()
        # sft layout: col = (g*8 + b)*3 + ch;  [128, 48]
        s = np.ascontiguousarray(
            sft3[sl].reshape(BL * 8, 128, IN).transpose(1, 0, 2)
            .reshape(128, BL * 8 * IN))
        c0 = F32_COLS["sft"][0]
        wc32[:, c0:c0 + BL * 8 * IN] = s
        c0 = F16_COLS["h0"][0]
        wc16[:, c0:c0 + BL * 8 * IN] = s.astype(f16)
        for g in range(BL):
            gg = c * BL + g
            c0, _ = F16_COLS[f"cr{g}"]
            wc16[:, c0:c0 + J] = candidate_np[gg].astype(f16)[None, :]
            c0, _ = F16_COLS[f"gp{g}"]
            wc16[:, c0:c0 + 8] = graph_pool[gg].reshape(8, 128).T.astype(f16)
            c0, _ = F32_COLS[f"fm{g}"]
            wc32[:2, c0:c0 + D] = fm3[gg].T
        # adjT pre-shuffled to the exact SBUF layout: row p holds, for each
        # half hf and block b, AT[512*hf + 128*b + p, :]  (AT = adj.T)
        a = adj[sl].transpose(0, 2, 1).reshape(BL, 2, 4, 128, N)
        a = a.transpose(0, 3, 1, 2, 4).reshape(BL, 128, 8192)
        in_maps.append({
            "wf16": wc16, "wf32": wc32,
            "adjT": np.ascontiguousarray(a).astype(f16),
        })
    return in_maps


def _install_ntff_hook():
    import sys, types
    if "antenv.axon_hooks" in sys.modules:
        return
    try:
        import antenv
        mod = types.ModuleType("antenv.axon_hooks")
        mod._hook = None
        mod.set_axon_ntff_profile_hook = lambda h: setattr(mod, "_hook", h)
        mod.get_axon_ntff_profile_hook = lambda: mod._hook
        sys.modules["antenv.axon_hooks"] = mod
        antenv.axon_hooks = mod
        from trn_agent_boot.trn_boot import _ntff_profile_via_ctypes
        mod.set_axon_ntff_profile_hook(
            _ntff_profile_via_ctypes("/opt/axon/libaxon_pjrt.so"))
    except Exception:
        pass


def _gumbel_noise():
    if "gumbel" not in _cache:
        import jax
        k1, k2 = jax.random.split(jax.random.key(42))
        g1 = np.asarray(jax.random.gumbel(k1, (B, J), np.float32))
        g2 = np.asarray(jax.random.gumbel(k2, (B, D), np.float32))
        _cache["gumbel"] = (g1, g2)
    return _cache["gumbel"]


def _run_device(in_maps, trace=False):
    from concourse.bass_utils import run_bass_kernel_spmd
    if "nc" not in _cache:
        _cache["nc"] = _build_nc()
    if trace:
        _install_ntff_hook()
    res = run_bass_kernel_spmd(
        _cache["nc"], in_maps, core_ids=list(range(NCORES)), trace=trace)
    O = np.concatenate([r["outs"] for r in res.results], 0)   # (B, 161)
    S = O[:, 0:J]
    V = O[:, J:J + 1]
    DS = O[:, J + 1:J + 1 + D]
    CR = O[:, J + 1 + D:J + 1 + 2 * D]
    return (S, V, DS, CR), res


def _host_tail(S, V, DS, CR, candidate, mask):
    mask = np.asarray(mask, bool)
    cand = np.asarray(candidate)
    g1, g2 = _gumbel_noise()
    logits = np.where(mask, -np.inf, S)
    zmax = np.max(logits, axis=1, keepdims=True)
    e = np.exp(logits - zmax)
    e[~np.isfinite(logits)] = 0.0
    esum = e.sum(1, keepdims=True)
    pi = (e / esum)[..., None].astype(np.float32)
    task_ix = np.argmax(logits + g1, axis=1).astype(np.int32)
    lse = zmax[:, 0] + np.log(esum[:, 0])
    logp = (np.take_along_axis(logits, task_ix[:, None].astype(np.int64), 1)[:, 0]
            - lse).astype(np.float32)
    sel = np.take_along_axis(cand, task_ix[:, None].astype(np.int64), 1)[:, 0]
    sel = sel.astype(np.int32)
    dmax = DS.max(1, keepdims=True)
    ed = np.exp(DS - dmax)
    edsum = ed.sum(1, keepdims=True)
    mhi = (ed / edsum)[..., None].astype(np.float32)
    device_ID = np.argmax(DS + g2, axis=1).astype(np.int32)
    mh_logp = (np.take_along_axis(DS, device_ID[:, None].astype(np.int64), 1)[:, 0]
               - (dmax[:, 0] + np.log(edsum[:, 0]))).astype(np.float32)
    vm = CR.min(1).astype(np.float32)
    v = V.astype(np.float32).reshape(B, 1)
    return (sel, task_ix, pi, v, logp, device_ID, mhi, vm, mh_logp)


def kernel(state_ft, state_fm, candidate, mask, adj, graph_pool, params):
    in_maps = _prep_in_maps(state_ft, state_fm, candidate, mask, adj,
                            graph_pool, params)
    (S, V, DS, CR), _ = _run_device(in_maps)
    return _host_tail(S, V, DS, CR, candidate, mask)


# revision 16
# speedup vs baseline: 1.1855x; 1.0193x over previous
"""Trainium2 Bass kernel for nn_ActorCritic (GIN actor-critic forward pass).

Sharding: data-parallel over the batch dim B=16 -> 2 graphs per core on 8
NeuronCores. Weights replicated. No collectives.

Device math (per graph, feature-major GNN, fp16 matmuls / f32 accumulate):
  q  = h.T @ A.T       (lhsT = h node-major; rhs = adjT, uploaded pre-shuffled
                        into the exact SBUF layout so the DMA is contiguous)
  z1 = W1.T @ q        -> t = relu(z1 + b1)   (ACT, per-partition bias)
  z2 = t.T @ W2 + b2   (node-major: t-slices stationary, bias via PSUM-prefill
                        matmul) -> h = relu(z2) on DVE
  pool + candidate gather fused in one matmul: rhs = [gp | onehot(cand)].
  actor/critic heads fp16; device (scatter) branch fp32 end-to-end.
Host: blob-packs weights/constants (2 DMAs), softmax/sampling tail (gumbel
trick with the reference's jax keys).
"""
import numpy as np

B, N, J, D, H, HA, HC, IN = 16, 1024, 128, 16, 128, 64, 64, 3
NCORES = 8
BL = B // NCORES  # graphs per core

_cache = {}

# ---- f16 blob column layout (128-row items only) ----
F16_COLS = {}
_c = 0
def _f16(name, cols):
    global _c
    F16_COLS[name] = (_c, cols)
    _c += cols
for _l in range(3):
    _f16(f"g{_l}W1", H)
for _l in range(3):
    _f16(f"g{_l}W2", H)
_f16("aW1t", HA); _f16("aW1b", HA); _f16("aW2", 1)
_f16("cW1", HC); _f16("cW2", 1)
_f16("h0", BL * 8 * IN)
_f16("cr0", J); _f16("cr1", J)
_f16("gp0", 8); _f16("gp1", 8)
C16 = _c

# ---- wrow: 4-row fp16 param: rows 0..2 = b2rep per layer, row 3 = ones ----
CROW = N

# ---- f32 hot blob ----
F32_COLS = {}
_c = 0
def _f32(name, cols):
    global _c
    F32_COLS[name] = (_c, cols)
    _c += cols
_f32("sft", BL * 8 * IN)
_f32("fm0", D); _f32("fm1", D)
_f32("iota16", D)
_f32("iota8", 8)
for _l in range(3):
    _f32(f"g{_l}b1", 1)
_f32("ab1", 1); _f32("ab2", 1); _f32("cb1", 1); _f32("cb2", 1)
_f32("aplW1hw", HA); _f32("cplW1hw", HA)
_f32("aplb1", 1); _f32("aplW2", 1); _f32("aplb2", 1)
_f32("cplb1", 1); _f32("cplW2", 1); _f32("cplb2", 1)
C32 = _c

# ---- f32 cold blob: the two 1024-col W1e matrices ----
F32B_COLS = {"aplW1e": (0, 8 * HA), "cplW1e": (8 * HA, 8 * HA)}
C32B = 16 * HA


def _build_nc():
    import concourse.mybir as mybir
    from concourse import bacc
    from concourse.tile import TileContext

    f32 = mybir.dt.float32
    f16 = mybir.dt.float16
    AF = mybir.ActivationFunctionType
    ALU = mybir.AluOpType

    nc = bacc.Bacc("TRN2", target_bir_lowering=False, debug=False)

    wf16_e = nc.declare_dram_parameter("wf16", [128, C16], f16, isOutput=False)
    wrow_e = nc.declare_dram_parameter("wrow", [4, CROW], f16, isOutput=False)
    wf32_e = nc.declare_dram_parameter("wf32", [128, C32], f32, isOutput=False)
    wf32b_e = nc.declare_dram_parameter("wf32b", [128, C32B], f32, isOutput=False)
    adjT_e = nc.declare_dram_parameter("adjT", [BL, 128, 8192], f16, isOutput=False)
    out_e = nc.declare_dram_parameter("outs", [BL, 161], f32, isOutput=True)

    with TileContext(nc) as tc:
        with tc.tile_pool(name="sb", bufs=1) as sb, \
             tc.tile_pool(name="ps", bufs=1, space="PSUM") as ps:

            wf16 = sb.tile([128, C16], f16, tag="wf16")
            nc.sync.dma_start(wf16[:], wf16_e[:])
            wf32 = sb.tile([128, C32], f32, tag="wf32")
            nc.sync.dma_start(wf32[:], wf32_e[:])
            b2r = []
            for l in range(3):
                r_ = sb.tile([1, CROW], f16, tag=f"b2r{l}", name=f"b2r{l}")
                nc.sync.dma_start(r_[:], wrow_e[l:l + 1, :])
                b2r.append(r_)
            ones1 = sb.tile([1, 128], f16, tag="ones1")
            nc.sync.dma_start(ones1[:], wrow_e[3:4, 0:128])
            wf32b = sb.tile([128, C32B], f32, tag="wf32b")

            def W16(name, rows=128):
                c0, cn = F16_COLS[name]
                return wf16[:rows, c0:c0 + cn]

            def W32(name, rows=128):
                c0, cn = F32_COLS[name]
                return wf32[:rows, c0:c0 + cn]

            def W32B(name, rows=128):
                c0, cn = F32B_COLS[name]
                return wf32b[:rows, c0:c0 + cn]

            At = []
            for g in range(BL):
                t_ = sb.tile([128, 8192], f16, tag=f"at{g}", name=f"at{g}")
                for qt in range(4):
                    nc.sync.dma_start(t_[:, qt * 2048:(qt + 1) * 2048],
                                      adjT_e[g, :, qt * 2048:(qt + 1) * 2048])
                    if g == 0 and qt == 1:
                        nc.sync.dma_start(wf32b[:], wf32b_e[:])
                At.append(t_)

            out_sb = []
            for g in range(BL):
                o_ = sb.tile([1, 161], f32, tag=f"out{g}", name=f"out{g}")
                out_sb.append(o_)

            # ---- prebuild gather rhs tiles (only need the blobs) ----
            Rt = {}
            for g in range(BL):
                for jb in range(8):
                    R = sb.tile([128, 1 + J], f16, tag=f"R{g}_{jb}",
                                name=f"R{g}_{jb}")
                    nc.vector.tensor_copy(R[:, 0:1], W16(f"gp{g}")[:, jb:jb + 1])
                    nc.vector.tensor_scalar(
                        R[:, 1:1 + J], W16(f"cr{g}"), W32("iota8")[:, jb:jb + 1],
                        None, ALU.is_equal)
                    Rt[g, jb] = R

            # ---- device (scatter) branch, fp32 — runs under the adjT DMAs
            for g in range(BL):
                Mt = []
                for jb in range(8):
                    m_ = sb.tile([128, D], f32, tag=f"mt{g}_{jb}", name=f"mt{g}_{jb}")
                    gib = g * 8 + jb
                    base = F32_COLS["sft"][0] + gib * IN
                    nc.vector.tensor_scalar(
                        m_[:], W32("iota16"), wf32[:, base + 2:base + 3],
                        wf32[:, base:base + 1], ALU.is_equal, ALU.mult)
                    Mt.append(m_)
                for p, osl in (("apl", slice(129, 145)), ("cpl", slice(145, 161))):
                    y1 = ps.tile([HA, D], f32, tag="z1c", bufs=2, name=f"y1{g}{p}")
                    w1e = W32B(f"{p}W1e")
                    for jb in range(8):
                        nc.tensor.matmul(y1[:], w1e[:, jb * HA:(jb + 1) * HA],
                                         Mt[jb][:], start=(jb == 0), stop=False)
                    nc.tensor.matmul(y1[:], W32(f"{p}W1hw", 2),
                                     W32(f"fm{g}", 2), start=False, stop=True)
                    tpl = sb.tile([HA, D], f32, tag=f"tpl{g}_{p}", name=f"tpl{g}_{p}")
                    nc.scalar.activation(tpl[:], y1[:], AF.Tanh,
                                         bias=W32(f"{p}b1", HA))
                    yp = ps.tile([1, D], f32, tag="z2c", bufs=3, name=f"yp{g}{p}")
                    nc.tensor.matmul(yp[:], W32(f"{p}W2", HA), tpl[:],
                                     start=True, stop=True)
                    nc.scalar.activation(out_sb[g][:, osl], yp[:], AF.Identity,
                                         bias=W32(f"{p}b2", 1))

            # ---- GNN, chunk-pipelined, graphs interleaved ----
            def At_rhs(g, jb, lo, width):
                base = (jb // 4) * 4096 + (jb % 4) * N
                return At[g][:, base + lo: base + lo + width]

            h_nm = {g: None for g in range(BL)}
            for l in range(3):
                din = IN if l == 0 else H
                qb, t_fm, h_new = {}, {}, {}
                for g in range(BL):
                    qb[g] = sb.tile([din, N], f16, tag="qb", bufs=3,
                                    name=f"qb{g}_{l}")
                    t_fm[g] = sb.tile([128, N], f16, tag="t", bufs=3,
                                      name=f"t{g}_{l}")
                    h_new[g] = sb.tile([128, N], f16, tag="h", bufs=4,
                                       name=f"h{g}_{l}")
                for c in range(2):
                    for g in range(BL):
                        cs = slice(c * 512, (c + 1) * 512)
                        q = ps.tile([128, 512], f32, tag="qc", bufs=3,
                                    name=f"q{g}_{l}_{c}")
                        for jb in range(8):
                            if l == 0:
                                b0 = F16_COLS["h0"][0] + (g * 8 + jb) * IN
                                lhsT = wf16[:, b0:b0 + IN]
                            else:
                                lhsT = h_nm[g][:, jb * H:(jb + 1) * H]
                            nc.tensor.matmul(q[:din, :], lhsT,
                                             At_rhs(g, jb, c * 512, 512),
                                             start=(jb == 0), stop=(jb == 7))
                        nc.vector.tensor_copy(qb[g][:, cs], q[:din, :])
                        z1 = ps.tile([128, 512], f32, tag="z1c", bufs=2,
                                     name=f"z1{g}_{l}_{c}")
                        nc.tensor.matmul(z1[:], W16(f"g{l}W1", din),
                                         qb[g][:, cs], start=True, stop=True)
                        nc.scalar.activation(t_fm[g][:, cs], z1[:], AF.Relu,
                                             bias=W32(f"g{l}b1", H))
                        z2 = ps.tile([128, 512], f32, tag="z2c", bufs=3,
                                     name=f"z2{g}_{l}_{c}")
                        nc.tensor.matmul(z2[:], ones1[:],
                                         b2r[l][:, cs],
                                         start=True, stop=False)
                        for k in range(4):
                            ib = 4 * c + k
                            nc.tensor.matmul(
                                z2[:, k * H:(k + 1) * H],
                                t_fm[g][:, ib * H:(ib + 1) * H],
                                W16(f"g{l}W2", H), start=False, stop=True)
                        nc.vector.tensor_scalar(h_new[g][:, cs], z2[:], 0.0,
                                                None, ALU.max)
                for g in range(BL):
                    h_nm[g] = h_new[g]

            for g in range(BL):
                # ---- pool + candidate gather ----
                P = ps.tile([128, 1 + J], f32, tag="z1c", bufs=2, name=f"P{g}")
                for jb in range(8):
                    nc.tensor.matmul(P[:], h_nm[g][:, jb * H:(jb + 1) * H],
                                     Rt[g, jb][:], start=(jb == 0),
                                     stop=(jb == 7))
                Pb = sb.tile([128, 1 + J], f16, tag=f"Pb{g}")
                nc.vector.tensor_copy(Pb[:], P[:])

                # ---- actor head ----
                zcf = ps.tile([HA, J], f32, tag="z2c", bufs=3, name=f"zcf{g}")
                nc.tensor.matmul(zcf[:], W16("aW1t"), Pb[:, 1:1 + J],
                                 start=True, stop=True)
                zhp = ps.tile([HA, 1], f32, tag="qc", bufs=3, name=f"zhp{g}")
                nc.tensor.matmul(zhp[:], W16("aW1b"), Pb[:, 0:1],
                                 start=True, stop=True)
                bias_a = sb.tile([HA, 1], f32, tag=f"biasa{g}")
                nc.vector.tensor_add(bias_a[:], zhp[:], W32("ab1", HA))
                ta = sb.tile([HA, J], f16, tag=f"ta{g}")
                nc.scalar.activation(ta[:], zcf[:], AF.Tanh, bias=bias_a[:])
                scp = ps.tile([1, J], f32, tag="qc", bufs=3, name=f"scp{g}")
                nc.tensor.matmul(scp[:], W16("aW2", HA), ta[:],
                                 start=True, stop=True)
                nc.scalar.activation(out_sb[g][:, 0:J], scp[:], AF.Identity,
                                     bias=W32("ab2", 1))

                # ---- critic head ----
                zv = ps.tile([HC, 1], f32, tag="qc", bufs=3, name=f"zv{g}")
                nc.tensor.matmul(zv[:], W16("cW1"), Pb[:, 0:1],
                                 start=True, stop=True)
                tv = sb.tile([HC, 1], f16, tag=f"tv{g}")
                nc.scalar.activation(tv[:], zv[:], AF.Tanh, bias=W32("cb1", HC))
                vv = ps.tile([1, 1], f32, tag="qc", bufs=3, name=f"vv{g}")
                nc.tensor.matmul(vv[:], W16("cW2", HC), tv[:],
                                 start=True, stop=True)
                nc.scalar.activation(out_sb[g][:, J:J + 1], vv[:], AF.Identity,
                                     bias=W32("cb2", 1))
                nc.sync.dma_start(out_e[g:g + 1, :], out_sb[g][:])

    nc.compile()
    return nc


def _flatten_params(params):
    out = {}
    for l, layer in enumerate(params["gnn"]):
        for k, v in layer.items():
            out[f"gnn{l}_{k}"] = v
    for head in ("actor", "critic", "actorPL", "criticPL"):
        for k, v in params[head].items():
            out[f"{head}_{k}"] = v
    return out


def _prep_in_maps(state_ft, state_fm, candidate, mask, adj, graph_pool, params):
    f16 = np.float16
    state_ft = np.asarray(state_ft, np.float32)
    state_fm = np.asarray(state_fm, np.float32)
    candidate_np = np.asarray(candidate)
    adj = np.asarray(adj, np.float32)
    graph_pool = np.asarray(graph_pool, np.float32)
    P = {k: np.asarray(v, np.float32) for k, v in _flatten_params(params).items()}

    w16 = np.zeros((128, C16), f16)
    w32 = np.zeros((128, C32), np.float32)
    w32b = np.zeros((128, C32B), np.float32)
    wrow = np.zeros((4, CROW), f16)
    wrow[3, 0:128] = 1.0

    def put16(name, arr):
        c0, cn = F16_COLS[name]
        w16[:arr.shape[0], c0:c0 + arr.shape[1]] = arr.astype(f16)

    def put32(name, arr):
        c0, cn = F32_COLS[name]
        w32[:arr.shape[0], c0:c0 + arr.shape[1]] = arr

    for l in range(3):
        put16(f"g{l}W1", P[f"gnn{l}_W1"])
        put16(f"g{l}W2", P[f"gnn{l}_W2"])
        wrow[l, :] = np.tile(P[f"gnn{l}_b2"], 8).astype(f16)
        put32(f"g{l}b1", P[f"gnn{l}_b1"][:, None])
    put16("aW1t", P["actor_W1"][:H]); put16("aW1b", P["actor_W1"][H:])
    put16("aW2", P["actor_W2"]); put16("cW1", P["critic_W1"])
    put16("cW2", P["critic_W2"])
    put32("iota8", np.arange(8, dtype=np.float32)[None, :] * 128
          + np.arange(128, dtype=np.float32)[:, None])
    put32("ab1", P["actor_b1"][:, None]); put32("ab2", P["actor_b2"][:, None])
    put32("cb1", P["critic_b1"][:, None]); put32("cb2", P["critic_b2"][:, None])
    put32("iota16", np.broadcast_to(
        np.arange(D, dtype=np.float32), (128, D)).copy())
    for p, nm in (("apl", "actorPL"), ("cpl", "criticPL")):
        w1e = P[f"{nm}_W1"][2::2]                      # (N, HA)
        c0, cn = F32B_COLS[f"{p}W1e"]
        w32b[:, c0:c0 + cn] = np.ascontiguousarray(
            w1e.reshape(8, 128, HA).transpose(1, 0, 2).reshape(128, 8 * HA))
        put32(f"{p}W1hw", P[f"{nm}_W1"][:2])
        put32(f"{p}b1", P[f"{nm}_b1"][:, None])
        put32(f"{p}W2", P[f"{nm}_W2"])
        put32(f"{p}b2", P[f"{nm}_b2"][:, None])

    sft3 = state_ft.reshape(B, N, IN)
    fm3 = state_fm.reshape(B, D, 2)
    in_maps = []
    for c in range(NCORES):
        sl = slice(c * BL, (c + 1) * BL)
        wc16 = w16.copy()
        wc32 = w32.copy()
        # sft layout: col = (g*8 + b)*3 + ch;  [128, 48]
        s = np.ascontiguousarray(
            sft3[sl].reshape(BL * 8, 128, IN).transpose(1, 0, 2)
            .reshape(128, BL * 8 * IN))
        c0 = F32_COLS["sft"][0]
        wc32[:, c0:c0 + BL * 8 * IN] = s
        c0 = F16_COLS["h0"][0]
        wc16[:, c0:c0 + BL * 8 * IN] = s.astype(f16)
        for g in range(BL):
            gg = c * BL + g
            c0, _ = F16_COLS[f"cr{g}"]
            wc16[:, c0:c0 + J] = candidate_np[gg].astype(f16)[None, :]
            c0, _ = F16_COLS[f"gp{g}"]
            wc16[:, c0:c0 + 8] = graph_pool[gg].reshape(8, 128).T.astype(f16)
            c0, _ = F32_COLS[f"fm{g}"]
            wc32[:2, c0:c0 + D] = fm3[gg].T
        # adjT pre-shuffled to the exact SBUF layout: row p holds, for each
        # half hf and block b, AT[512*hf + 128*b + p, :]  (AT = adj.T)
        a = adj[sl].transpose(0, 2, 1).reshape(BL, 2, 4, 128, N)
        a = a.transpose(0, 3, 1, 2, 4).reshape(BL, 128, 8192)
        in_maps.append({
            "wf16": wc16, "wf32": wc32, "wf32b": w32b, "wrow": wrow,
            "adjT": np.ascontiguousarray(a).astype(f16),
        })
    return in_maps


def _install_ntff_hook():
    import sys, types
    if "antenv.axon_hooks" in sys.modules:
        return
    try:
        import antenv
        mod = types.ModuleType("antenv.axon_hooks")
        mod._hook = None
        mod.set_axon_ntff_profile_hook = lambda h: setattr(mod, "_hook", h)
        mod.get_axon_ntff_profile_hook = lambda: mod._hook
        sys.modules["antenv.axon_hooks"] = mod
        antenv.axon_hooks = mod
        from trn_agent_boot.trn_boot import _ntff_profile_via_ctypes
        mod.set_axon_ntff_profile_hook(
            _ntff_profile_via_ctypes("/opt/axon/libaxon_pjrt.so"))
    except Exception:
        pass


def _gumbel_noise():
    if "gumbel" not in _cache:
        import jax
        k1, k2 = jax.random.split(jax.random.key(42))
        g1 = np.asarray(jax.random.gumbel(k1, (B, J), np.float32))
        g2 = np.asarray(jax.random.gumbel(k2, (B, D), np.float32))
        _cache["gumbel"] = (g1, g2)
    return _cache["gumbel"]


def _run_device(in_maps, trace=False):
    from concourse.bass_utils import run_bass_kernel_spmd
    if "nc" not in _cache:
        _cache["nc"] = _build_nc()
    if trace:
        _install_ntff_hook()
    res = run_bass_kernel_spmd(
        _cache["nc"], in_maps, core_ids=list(range(NCORES)), trace=trace)
    O = np.concatenate([r["outs"] for r in res.results], 0)   # (B, 161)
    S = O[:, 0:J]
    V = O[:, J:J + 1]
    DS = O[:, J + 1:J + 1 + D]
    CR = O[:, J + 1 + D:J + 1 + 2 * D]
    return (S, V, DS, CR), res


def _host_tail(S, V, DS, CR, candidate, mask):
    mask = np.asarray(mask, bool)
    cand = np.asarray(candidate)
    g1, g2 = _gumbel_noise()
    logits = np.where(mask, -np.inf, S)
    zmax = np.max(logits, axis=1, keepdims=True)
    e = np.exp(logits - zmax)
    e[~np.isfinite(logits)] = 0.0
    esum = e.sum(1, keepdims=True)
    pi = (e / esum)[..., None].astype(np.float32)
    task_ix = np.argmax(logits + g1, axis=1).astype(np.int32)
    lse = zmax[:, 0] + np.log(esum[:, 0])
    logp = (np.take_along_axis(logits, task_ix[:, None].astype(np.int64), 1)[:, 0]
            - lse).astype(np.float32)
    sel = np.take_along_axis(cand, task_ix[:, None].astype(np.int64), 1)[:, 0]
    sel = sel.astype(np.int32)
    dmax = DS.max(1, keepdims=True)
    ed = np.exp(DS - dmax)
    edsum = ed.sum(1, keepdims=True)
    mhi = (ed / edsum)[..., None].astype(np.float32)
    device_ID = np.argmax(DS + g2, axis=1).astype(np.int32)
    mh_logp = (np.take_along_axis(DS, device_ID[:, None].astype(np.int64), 1)[:, 0]
               - (dmax[:, 0] + np.log(edsum[:, 0]))).astype(np.float32)
    vm = CR.min(1).astype(np.float32)
    v = V.astype(np.float32).reshape(B, 1)
    return (sel, task_ix, pi, v, logp, device_ID, mhi, vm, mh_logp)


def kernel(state_ft, state_fm, candidate, mask, adj, graph_pool, params):
    in_maps = _prep_in_maps(state_ft, state_fm, candidate, mask, adj,
                            graph_pool, params)
    (S, V, DS, CR), _ = _run_device(in_maps)
    return _host_tail(S, V, DS, CR, candidate, mask)


# revision 17
# speedup vs baseline: 1.2233x; 1.0319x over previous
"""Trainium2 Bass kernel for nn_ActorCritic (GIN actor-critic forward pass).

Sharding: data-parallel over the batch dim B=16 -> 2 graphs per core on 8
NeuronCores. Weights replicated. No collectives.

Device math (per graph, feature-major GNN, fp16 matmuls / f32 accumulate):
  q  = h.T @ A.T       (lhsT = h node-major; rhs = adjT, uploaded pre-shuffled
                        into the exact SBUF layout so the DMA is contiguous)
  z1 = W1.T @ q        -> t = relu(z1 + b1)   (ACT, per-partition bias)
  z2 = t.T @ W2 + b2   (node-major: t-slices stationary, bias via PSUM-prefill
                        matmul) -> h = relu(z2) on DVE
  pool + candidate gather fused in one matmul: rhs = [gp | onehot(cand)].
  actor/critic heads fp16; device (scatter) branch fp32 end-to-end.
Host: blob-packs weights/constants (2 DMAs), softmax/sampling tail (gumbel
trick with the reference's jax keys).
"""
import numpy as np

B, N, J, D, H, HA, HC, IN = 16, 1024, 128, 16, 128, 64, 64, 3
NCORES = 8
BL = B // NCORES  # graphs per core

_cache = {}

# ---- f16 blob column layout (128-row items only) ----
F16_COLS = {}
_c = 0
def _f16(name, cols):
    global _c
    F16_COLS[name] = (_c, cols)
    _c += cols
for _l in range(3):
    _f16(f"g{_l}W1", H)
for _l in range(3):
    _f16(f"g{_l}W2", H)
_f16("aW1t", HA); _f16("aW1b", HA); _f16("aW2", 1)
_f16("cW1", HC); _f16("cW2", 1)
_f16("h0", BL * 8 * IN)
_f16("fm0", D); _f16("fm1", D)
_f16("plhw_a", HA); _f16("plhw_c", HA)
_f16("plW2_a", 1); _f16("plW2_c", 1)
_f16("cr0", J); _f16("cr1", J)
_f16("gp0", 8); _f16("gp1", 8)
C16 = _c

# ---- wrow: 4-row fp16 param: rows 0..2 = b2rep per layer, row 3 = ones ----
CROW = N

# ---- f32 hot blob ----
F32_COLS = {}
_c = 0
def _f32(name, cols):
    global _c
    F32_COLS[name] = (_c, cols)
    _c += cols
_f32("sft", BL * 8 * IN)
_f32("iota16", D)
_f32("iota8", 8)
for _l in range(3):
    _f32(f"g{_l}b1", 1)
_f32("ab1", 1); _f32("ab2", 1); _f32("cb1", 1); _f32("cb2", 1)
_f32("aplb1", 1); _f32("aplW2", 1); _f32("aplb2", 1)
_f32("cplb1", 1); _f32("cplW2", 1); _f32("cplb2", 1)
C32 = _c

# ---- f16 cold blob: the two 1024-col W1e matrices ----
F16B_COLS = {"aplW1e": (0, 8 * HA), "cplW1e": (8 * HA, 8 * HA)}
C16B = 16 * HA


def _build_nc():
    import concourse.mybir as mybir
    from concourse import bacc
    from concourse.tile import TileContext

    f32 = mybir.dt.float32
    f16 = mybir.dt.float16
    AF = mybir.ActivationFunctionType
    ALU = mybir.AluOpType

    nc = bacc.Bacc("TRN2", target_bir_lowering=False, debug=False)

    wf16_e = nc.declare_dram_parameter("wf16", [128, C16], f16, isOutput=False)
    wf32_e = nc.declare_dram_parameter("wf32", [128, C32], f32, isOutput=False)
    wf16b_e = nc.declare_dram_parameter("wf16b", [128, C16B], f16, isOutput=False)
    b2b_e = nc.declare_dram_parameter("b2b", [3, N], f32, isOutput=False)
    adjT_e = nc.declare_dram_parameter("adjT", [BL, 128, 8192], f16, isOutput=False)
    out_e = nc.declare_dram_parameter("outs", [BL, 161], f32, isOutput=True)

    with TileContext(nc) as tc:
        with tc.tile_pool(name="sb", bufs=1) as sb, \
             tc.tile_pool(name="ps", bufs=1, space="PSUM") as ps:

            wf16 = sb.tile([128, C16], f16, tag="wf16")
            nc.sync.dma_start(wf16[:], wf16_e[:])
            wf32 = sb.tile([128, C32], f32, tag="wf32")
            nc.sync.dma_start(wf32[:], wf32_e[:])
            wf16b = sb.tile([128, C16B], f16, tag="wf16b")
            b2rep = []
            for l in range(3):
                r_ = sb.tile([128, N], f32, tag=f"b2rep{l}", name=f"b2rep{l}")
                nc.sync.dma_start(r_[:], b2b_e[l].partition_broadcast(128))
                b2rep.append(r_)

            def W16(name, rows=128):
                c0, cn = F16_COLS[name]
                return wf16[:rows, c0:c0 + cn]

            def W32(name, rows=128):
                c0, cn = F32_COLS[name]
                return wf32[:rows, c0:c0 + cn]

            def W16B(name, rows=128):
                c0, cn = F16B_COLS[name]
                return wf16b[:rows, c0:c0 + cn]

            At = []
            for g in range(BL):
                t_ = sb.tile([128, 8192], f16, tag=f"at{g}", name=f"at{g}")
                for qt in range(4):
                    nc.sync.dma_start(t_[:, qt * 2048:(qt + 1) * 2048],
                                      adjT_e[g, :, qt * 2048:(qt + 1) * 2048])
                    if g == 0 and qt == 1:
                        nc.sync.dma_start(wf16b[:], wf16b_e[:])
                At.append(t_)

            out_sb = []
            for g in range(BL):
                o_ = sb.tile([1, 161], f32, tag=f"out{g}", name=f"out{g}")
                out_sb.append(o_)

            # ---- prebuild gather rhs tiles (only need the blobs) ----
            Rt = {}
            for g in range(BL):
                for jb in range(8):
                    R = sb.tile([128, 1 + J], f16, tag=f"R{g}_{jb}",
                                name=f"R{g}_{jb}")
                    nc.vector.tensor_copy(R[:, 0:1], W16(f"gp{g}")[:, jb:jb + 1])
                    nc.vector.tensor_scalar(
                        R[:, 1:1 + J], W16(f"cr{g}"), W32("iota8")[:, jb:jb + 1],
                        None, ALU.is_equal)
                    Rt[g, jb] = R

            # ---- device (scatter) branch, fp32 — runs under the adjT DMAs
            for g in range(BL):
                Mt = []
                for jb in range(8):
                    m_ = sb.tile([128, D], f16, tag=f"mt{g}_{jb}", name=f"mt{g}_{jb}")
                    gib = g * 8 + jb
                    base = F32_COLS["sft"][0] + gib * IN
                    nc.vector.tensor_scalar(
                        m_[:], W32("iota16"), wf32[:, base + 2:base + 3],
                        wf32[:, base:base + 1], ALU.is_equal, ALU.mult)
                    Mt.append(m_)
                for p, osl in (("apl", slice(129, 145)), ("cpl", slice(145, 161))):
                    y1 = ps.tile([HA, D], f32, tag="z1c", bufs=2, name=f"y1{g}{p}")
                    w1e = W16B(f"{p}W1e")
                    for jb in range(8):
                        nc.tensor.matmul(y1[:], w1e[:, jb * HA:(jb + 1) * HA],
                                         Mt[jb][:], start=(jb == 0), stop=False)
                    nc.tensor.matmul(y1[:], W16(f"plhw_{p[0]}", 2),
                                     W16(f"fm{g}", 2), start=False, stop=True)
                    tpl = sb.tile([HA, D], f16, tag=f"tpl{g}_{p}", name=f"tpl{g}_{p}")
                    nc.scalar.activation(tpl[:], y1[:], AF.Tanh,
                                         bias=W32(f"{p}b1", HA))
                    yp = ps.tile([1, D], f32, tag="z2c", bufs=3, name=f"yp{g}{p}")
                    nc.tensor.matmul(yp[:], W16(f"plW2_{p[0]}", HA), tpl[:],
                                     start=True, stop=True)
                    nc.scalar.activation(out_sb[g][:, osl], yp[:], AF.Identity,
                                         bias=W32(f"{p}b2", 1))

            # ---- GNN, chunk-pipelined, graphs interleaved ----
            def At_rhs(g, jb, lo, width):
                base = (jb // 4) * 4096 + (jb % 4) * N
                return At[g][:, base + lo: base + lo + width]

            h_nm = {g: None for g in range(BL)}
            for l in range(3):
                din = IN if l == 0 else H
                qb, t_fm, h_new = {}, {}, {}
                for g in range(BL):
                    qb[g] = sb.tile([din, N], f16, tag="qb", bufs=3,
                                    name=f"qb{g}_{l}")
                    t_fm[g] = sb.tile([128, N], f16, tag="t", bufs=3,
                                      name=f"t{g}_{l}")
                    h_new[g] = sb.tile([128, N], f16, tag="h", bufs=4,
                                       name=f"h{g}_{l}")
                for c in range(2):
                    for g in range(BL):
                        cs = slice(c * 512, (c + 1) * 512)
                        q = ps.tile([128, 512], f32, tag="qc", bufs=3,
                                    name=f"q{g}_{l}_{c}")
                        for jb in range(8):
                            if l == 0:
                                b0 = F16_COLS["h0"][0] + (g * 8 + jb) * IN
                                lhsT = wf16[:, b0:b0 + IN]
                            else:
                                lhsT = h_nm[g][:, jb * H:(jb + 1) * H]
                            nc.tensor.matmul(q[:din, :], lhsT,
                                             At_rhs(g, jb, c * 512, 512),
                                             start=(jb == 0), stop=(jb == 7))
                        nc.vector.tensor_copy(qb[g][:, cs], q[:din, :])
                        z1 = ps.tile([128, 512], f32, tag="z1c", bufs=2,
                                     name=f"z1{g}_{l}_{c}")
                        nc.tensor.matmul(z1[:], W16(f"g{l}W1", din),
                                         qb[g][:, cs], start=True, stop=True)
                        nc.scalar.activation(t_fm[g][:, cs], z1[:], AF.Relu,
                                             bias=W32(f"g{l}b1", H))
                        z2 = ps.tile([128, 512], f32, tag="z2c", bufs=3,
                                     name=f"z2{g}_{l}_{c}")
                        for k in range(4):
                            ib = 4 * c + k
                            nc.tensor.matmul(
                                z2[:, k * H:(k + 1) * H],
                                t_fm[g][:, ib * H:(ib + 1) * H],
                                W16(f"g{l}W2", H), start=True, stop=True)
                        ht = sb.tile([128, 512], f16, tag="ht", bufs=3,
                                     name=f"ht{g}_{l}_{c}")
                        nc.vector.tensor_add(ht[:], z2[:], b2rep[l][:, cs])
                        nc.scalar.activation(h_new[g][:, cs], ht[:], AF.Relu)
                for g in range(BL):
                    h_nm[g] = h_new[g]

            for g in range(BL):
                # ---- pool + candidate gather ----
                P = ps.tile([128, 1 + J], f32, tag="z1c", bufs=2, name=f"P{g}")
                for jb in range(8):
                    nc.tensor.matmul(P[:], h_nm[g][:, jb * H:(jb + 1) * H],
                                     Rt[g, jb][:], start=(jb == 0),
                                     stop=(jb == 7))
                Pb = sb.tile([128, 1 + J], f16, tag=f"Pb{g}")
                nc.vector.tensor_copy(Pb[:], P[:])

                # ---- actor head ----
                zcf = ps.tile([HA, J], f32, tag="z2c", bufs=3, name=f"zcf{g}")
                nc.tensor.matmul(zcf[:], W16("aW1t"), Pb[:, 1:1 + J],
                                 start=True, stop=True)
                zhp = ps.tile([HA, 1], f32, tag="qc", bufs=3, name=f"zhp{g}")
                nc.tensor.matmul(zhp[:], W16("aW1b"), Pb[:, 0:1],
                                 start=True, stop=True)
                bias_a = sb.tile([HA, 1], f32, tag=f"biasa{g}")
                nc.vector.tensor_add(bias_a[:], zhp[:], W32("ab1", HA))
                ta = sb.tile([HA, J], f16, tag=f"ta{g}")
                nc.scalar.activation(ta[:], zcf[:], AF.Tanh, bias=bias_a[:])
                scp = ps.tile([1, J], f32, tag="qc", bufs=3, name=f"scp{g}")
                nc.tensor.matmul(scp[:], W16("aW2", HA), ta[:],
                                 start=True, stop=True)
                nc.scalar.activation(out_sb[g][:, 0:J], scp[:], AF.Identity,
                                     bias=W32("ab2", 1))

                # ---- critic head ----
                zv = ps.tile([HC, 1], f32, tag="qc", bufs=3, name=f"zv{g}")
                nc.tensor.matmul(zv[:], W16("cW1"), Pb[:, 0:1],
                                 start=True, stop=True)
                tv = sb.tile([HC, 1], f16, tag=f"tv{g}")
                nc.scalar.activation(tv[:], zv[:], AF.Tanh, bias=W32("cb1", HC))
                vv = ps.tile([1, 1], f32, tag="qc", bufs=3, name=f"vv{g}")
                nc.tensor.matmul(vv[:], W16("cW2", HC), tv[:],
                                 start=True, stop=True)
                nc.scalar.activation(out_sb[g][:, J:J + 1], vv[:], AF.Identity,
                                     bias=W32("cb2", 1))
                nc.sync.dma_start(out_e[g:g + 1, :], out_sb[g][:])

    nc.compile()
    return nc


def _flatten_params(params):
    out = {}
    for l, layer in enumerate(params["gnn"]):
        for k, v in layer.items():
            out[f"gnn{l}_{k}"] = v
    for head in ("actor", "critic", "actorPL", "criticPL"):
        for k, v in params[head].items():
            out[f"{head}_{k}"] = v
    return out


def _prep_in_maps(state_ft, state_fm, candidate, mask, adj, graph_pool, params):
    f16 = np.float16
    state_ft = np.asarray(state_ft, np.float32)
    state_fm = np.asarray(state_fm, np.float32)
    candidate_np = np.asarray(candidate)
    adj = np.asarray(adj, np.float32)
    graph_pool = np.asarray(graph_pool, np.float32)
    P = {k: np.asarray(v, np.float32) for k, v in _flatten_params(params).items()}

    w16 = np.zeros((128, C16), f16)
    w32 = np.zeros((128, C32), np.float32)
    w16b = np.zeros((128, C16B), f16)
    b2b = np.zeros((3, N), np.float32)

    def put16(name, arr):
        c0, cn = F16_COLS[name]
        w16[:arr.shape[0], c0:c0 + arr.shape[1]] = arr.astype(f16)

    def put32(name, arr):
        c0, cn = F32_COLS[name]
        w32[:arr.shape[0], c0:c0 + arr.shape[1]] = arr

    for l in range(3):
        put16(f"g{l}W1", P[f"gnn{l}_W1"])
        put16(f"g{l}W2", P[f"gnn{l}_W2"])
        b2b[l, :] = np.tile(P[f"gnn{l}_b2"], 8)
        put32(f"g{l}b1", P[f"gnn{l}_b1"][:, None])
    put16("aW1t", P["actor_W1"][:H]); put16("aW1b", P["actor_W1"][H:])
    put16("aW2", P["actor_W2"]); put16("cW1", P["critic_W1"])
    put16("cW2", P["critic_W2"])
    put32("iota8", np.arange(8, dtype=np.float32)[None, :] * 128
          + np.arange(128, dtype=np.float32)[:, None])
    put32("ab1", P["actor_b1"][:, None]); put32("ab2", P["actor_b2"][:, None])
    put32("cb1", P["critic_b1"][:, None]); put32("cb2", P["critic_b2"][:, None])
    put32("iota16", np.broadcast_to(
        np.arange(D, dtype=np.float32), (128, D)).copy())
    for p, nm in (("apl", "actorPL"), ("cpl", "criticPL")):
        w1e = P[f"{nm}_W1"][2::2]                      # (N, HA)
        c0, cn = F16B_COLS[f"{p}W1e"]
        w16b[:, c0:c0 + cn] = np.ascontiguousarray(
            w1e.reshape(8, 128, HA).transpose(1, 0, 2)
            .reshape(128, 8 * HA)).astype(f16)
        put16(f"plhw_{p[0]}", P[f"{nm}_W1"][:2])
        put16(f"plW2_{p[0]}", P[f"{nm}_W2"])
        put32(f"{p}b1", P[f"{nm}_b1"][:, None])
        put32(f"{p}b2", P[f"{nm}_b2"][:, None])

    sft3 = state_ft.reshape(B, N, IN)
    fm3 = state_fm.reshape(B, D, 2)
    in_maps = []
    for c in range(NCORES):
        sl = slice(c * BL, (c + 1) * BL)
        wc16 = w16.copy()
        wc32 = w32.copy()
        # sft layout: col = (g*8 + b)*3 + ch;  [128, 48]
        s = np.ascontiguousarray(
            sft3[sl].reshape(BL * 8, 128, IN).transpose(1, 0, 2)
            .reshape(128, BL * 8 * IN))
        c0 = F32_COLS["sft"][0]
        wc32[:, c0:c0 + BL * 8 * IN] = s
        c0 = F16_COLS["h0"][0]
        wc16[:, c0:c0 + BL * 8 * IN] = s.astype(f16)
        for g in range(BL):
            gg = c * BL + g
            c0, _ = F16_COLS[f"cr{g}"]
            wc16[:, c0:c0 + J] = candidate_np[gg].astype(f16)[None, :]
            c0, _ = F16_COLS[f"gp{g}"]
            wc16[:, c0:c0 + 8] = graph_pool[gg].reshape(8, 128).T.astype(f16)
            c0, _ = F16_COLS[f"fm{g}"]
            wc16[:2, c0:c0 + D] = fm3[gg].T.astype(f16)
        # adjT pre-shuffled to the exact SBUF layout: row p holds, for each
        # half hf and block b, AT[512*hf + 128*b + p, :]  (AT = adj.T)
        a = adj[sl].transpose(0, 2, 1).reshape(BL, 2, 4, 128, N)
        a = a.transpose(0, 3, 1, 2, 4).reshape(BL, 128, 8192)
        in_maps.append({
            "wf16": wc16, "wf32": wc32, "wf16b": w16b, "b2b": b2b,
            "adjT": np.ascontiguousarray(a).astype(f16),
        })
    return in_maps


def _install_ntff_hook():
    import sys, types
    if "antenv.axon_hooks" in sys.modules:
        return
    try:
        import antenv
        mod = types.ModuleType("antenv.axon_hooks")
        mod._hook = None
        mod.set_axon_ntff_profile_hook = lambda h: setattr(mod, "_hook", h)
        mod.get_axon_ntff_profile_hook = lambda: mod._hook
        sys.modules["antenv.axon_hooks"] = mod
        antenv.axon_hooks = mod
        from trn_agent_boot.trn_boot import _ntff_profile_via_ctypes
        mod.set_axon_ntff_profile_hook(
            _ntff_profile_via_ctypes("/opt/axon/libaxon_pjrt.so"))
    except Exception:
        pass


def _gumbel_noise():
    if "gumbel" not in _cache:
        import jax
        k1, k2 = jax.random.split(jax.random.key(42))
        g1 = np.asarray(jax.random.gumbel(k1, (B, J), np.float32))
        g2 = np.asarray(jax.random.gumbel(k2, (B, D), np.float32))
        _cache["gumbel"] = (g1, g2)
    return _cache["gumbel"]


def _run_device(in_maps, trace=False):
    from concourse.bass_utils import run_bass_kernel_spmd
    if "nc" not in _cache:
        _cache["nc"] = _build_nc()
    if trace:
        _install_ntff_hook()
    res = run_bass_kernel_spmd(
        _cache["nc"], in_maps, core_ids=list(range(NCORES)), trace=trace)
    O = np.concatenate([r["outs"] for r in res.results], 0)   # (B, 161)
    S = O[:, 0:J]
    V = O[:, J:J + 1]
    DS = O[:, J + 1:J + 1 + D]
    CR = O[:, J + 1 + D:J + 1 + 2 * D]
    return (S, V, DS, CR), res


def _host_tail(S, V, DS, CR, candidate, mask):
    mask = np.asarray(mask, bool)
    cand = np.asarray(candidate)
    g1, g2 = _gumbel_noise()
    logits = np.where(mask, -np.inf, S)
    zmax = np.max(logits, axis=1, keepdims=True)
    e = np.exp(logits - zmax)
    e[~np.isfinite(logits)] = 0.0
    esum = e.sum(1, keepdims=True)
    pi = (e / esum)[..., None].astype(np.float32)
    task_ix = np.argmax(logits + g1, axis=1).astype(np.int32)
    lse = zmax[:, 0] + np.log(esum[:, 0])
    logp = (np.take_along_axis(logits, task_ix[:, None].astype(np.int64), 1)[:, 0]
            - lse).astype(np.float32)
    sel = np.take_along_axis(cand, task_ix[:, None].astype(np.int64), 1)[:, 0]
    sel = sel.astype(np.int32)
    dmax = DS.max(1, keepdims=True)
    ed = np.exp(DS - dmax)
    edsum = ed.sum(1, keepdims=True)
    mhi = (ed / edsum)[..., None].astype(np.float32)
    device_ID = np.argmax(DS + g2, axis=1).astype(np.int32)
    mh_logp = (np.take_along_axis(DS, device_ID[:, None].astype(np.int64), 1)[:, 0]
               - (dmax[:, 0] + np.log(edsum[:, 0]))).astype(np.float32)
    vm = CR.min(1).astype(np.float32)
    v = V.astype(np.float32).reshape(B, 1)
    return (sel, task_ix, pi, v, logp, device_ID, mhi, vm, mh_logp)


def kernel(state_ft, state_fm, candidate, mask, adj, graph_pool, params):
    in_maps = _prep_in_maps(state_ft, state_fm, candidate, mask, adj,
                            graph_pool, params)
    (S, V, DS, CR), _ = _run_device(in_maps)
    return _host_tail(S, V, DS, CR, candidate, mask)


# revision 20
# speedup vs baseline: 1.3595x; 1.1113x over previous
"""Trainium2 Bass kernel for nn_ActorCritic (GIN actor-critic forward pass).

Sharding: data-parallel over the batch dim B=16 -> 2 graphs per core on 8
NeuronCores. Weights replicated. No collectives.

Device math (per graph, feature-major GNN, fp16 matmuls / f32 accumulate):
  q  = h.T @ A.T       (lhsT = h node-major; rhs = adjT, uploaded pre-shuffled
                        into the exact SBUF layout so the DMA is contiguous)
  z1 = W1.T @ q        -> t = relu(z1 + b1)   (ACT, per-partition bias)
  z2 = t.T @ W2 + b2   (node-major: t-slices stationary, bias via PSUM-prefill
                        matmul) -> h = relu(z2) on DVE
  pool + candidate gather fused in one matmul: rhs = [gp | onehot(cand)].
  actor/critic heads fp16; device (scatter) branch fp32 end-to-end.
Host: blob-packs weights/constants (2 DMAs), softmax/sampling tail (gumbel
trick with the reference's jax keys).
"""
import numpy as np

B, N, J, D, H, HA, HC, IN = 16, 1024, 128, 16, 128, 64, 64, 3
NCORES = 8
BL = B // NCORES  # graphs per core

_cache = {}

# ---- f16 blob column layout (128-row items only) ----
F16_COLS = {}
_c = 0
def _f16(name, cols):
    global _c
    F16_COLS[name] = (_c, cols)
    _c += cols
for _l in range(3):
    _f16(f"g{_l}W1", H)
for _l in range(3):
    _f16(f"g{_l}W2", H)
_f16("aW1t", HA); _f16("aW1b", HA); _f16("aW2", 1)
_f16("cW1", HC); _f16("cW2", 1)
_f16("h0", BL * 8 * IN)
_f16("fm0", D); _f16("fm1", D)
_f16("plhw_a", HA); _f16("plhw_c", HA)
_f16("plW2_a", 1); _f16("plW2_c", 1)
_f16("cr0", J); _f16("cr1", J)
_f16("gp0", 8); _f16("gp1", 8)
C16 = _c

# ---- wrow: 4-row fp16 param: rows 0..2 = b2rep per layer, row 3 = ones ----
CROW = N

# ---- f32 hot blob ----
F32_COLS = {}
_c = 0
def _f32(name, cols):
    global _c
    F32_COLS[name] = (_c, cols)
    _c += cols
_f32("sft", BL * 8 * IN)
_f32("iota16", D)
_f32("iota8", 8)
for _l in range(3):
    _f32(f"g{_l}b1", 1)
_f32("ab1", 1); _f32("ab2", 1); _f32("cb1", 1); _f32("cb2", 1)
_f32("aplb1", 1); _f32("aplW2", 1); _f32("aplb2", 1)
_f32("cplb1", 1); _f32("cplW2", 1); _f32("cplb2", 1)
C32 = _c

# ---- f16 cold blob: the two 1024-col W1e matrices ----
F16B_COLS = {"aplW1e": (0, 8 * HA), "cplW1e": (8 * HA, 8 * HA)}
C16B = 16 * HA


def _build_nc():
    import concourse.mybir as mybir
    from concourse import bacc
    from concourse.tile import TileContext

    f32 = mybir.dt.float32
    f16 = mybir.dt.float16
    AF = mybir.ActivationFunctionType
    ALU = mybir.AluOpType

    nc = bacc.Bacc("TRN2", target_bir_lowering=False, debug=False)

    wf16_e = nc.declare_dram_parameter("wf16", [128, C16], f16, isOutput=False)
    wf32_e = nc.declare_dram_parameter("wf32", [128, C32], f32, isOutput=False)
    wf16b_e = nc.declare_dram_parameter("wf16b", [128, C16B], f16, isOutput=False)
    b2b_e = nc.declare_dram_parameter("b2b", [3, N], f32, isOutput=False)
    adjT_e = nc.declare_dram_parameter("adjT", [BL, 128, 8192], f16, isOutput=False)
    out_e = nc.declare_dram_parameter("outs", [BL, 161], f32, isOutput=True)

    with TileContext(nc) as tc:
        with tc.tile_pool(name="sb", bufs=1) as sb, \
             tc.tile_pool(name="ps", bufs=1, space="PSUM") as ps:

            warm = sb.tile([128, 512], f16, tag="warm")
            nc.vector.memset(warm[:], 0.0)
            wps = ps.tile([128, 512], f32, tag="qc", bufs=3, name="warmps")
            for wi in range(8):
                nc.tensor.matmul(wps[:], warm[:, 0:128], warm[:],
                                 start=(wi == 0), stop=(wi == 7))

            wf16 = sb.tile([128, C16], f16, tag="wf16")
            nc.sync.dma_start(wf16[:], wf16_e[:])
            wf32 = sb.tile([128, C32], f32, tag="wf32")
            nc.sync.dma_start(wf32[:], wf32_e[:])
            wf16b = sb.tile([128, C16B], f16, tag="wf16b")
            nc.sync.dma_start(wf16b[:], wf16b_e[:])
            b2rep = []
            for l in range(3):
                r_ = sb.tile([128, N], f32, tag=f"b2rep{l}", name=f"b2rep{l}")
                b2rep.append(r_)

            def W16(name, rows=128):
                c0, cn = F16_COLS[name]
                return wf16[:rows, c0:c0 + cn]

            def W32(name, rows=128):
                c0, cn = F32_COLS[name]
                return wf32[:rows, c0:c0 + cn]

            def W16B(name, rows=128):
                c0, cn = F16B_COLS[name]
                return wf16b[:rows, c0:c0 + cn]

            At = []
            for g in range(BL):
                t_ = sb.tile([128, 8192], f16, tag=f"at{g}", name=f"at{g}")
                for qt in range(4):
                    nc.sync.dma_start(t_[:, qt * 2048:(qt + 1) * 2048],
                                      adjT_e[g, :, qt * 2048:(qt + 1) * 2048])
                    l = {(0, 3): 0, (1, 1): 1, (1, 3): 2}.get((g, qt))
                    if l is not None:
                        nc.sync.dma_start(
                            b2rep[l][:], b2b_e[l].partition_broadcast(128))
                At.append(t_)

            out_sb = []
            for g in range(BL):
                o_ = sb.tile([1, 161], f32, tag=f"out{g}", name=f"out{g}")
                out_sb.append(o_)

            # ---- prebuild gather rhs tiles (only need the blobs) ----
            Rt = {}
            for g in range(BL):
                for jb in range(8):
                    R = sb.tile([128, 1 + J], f16, tag=f"R{g}_{jb}",
                                name=f"R{g}_{jb}")
                    nc.vector.tensor_copy(R[:, 0:1], W16(f"gp{g}")[:, jb:jb + 1])
                    nc.vector.tensor_scalar(
                        R[:, 1:1 + J], W16(f"cr{g}"), W32("iota8")[:, jb:jb + 1],
                        None, ALU.is_equal)
                    Rt[g, jb] = R

            # ---- device (scatter) branch, fp32 — runs under the adjT DMAs
            for g in range(BL):
                Mt = []
                for jb in range(8):
                    m_ = sb.tile([128, D], f16, tag=f"mt{g}_{jb}", name=f"mt{g}_{jb}")
                    gib = g * 8 + jb
                    base = F32_COLS["sft"][0] + gib * IN
                    nc.vector.tensor_scalar(
                        m_[:], W32("iota16"), wf32[:, base + 2:base + 3],
                        wf32[:, base:base + 1], ALU.is_equal, ALU.mult)
                    Mt.append(m_)
                for p, osl in (("apl", slice(129, 145)), ("cpl", slice(145, 161))):
                    y1 = ps.tile([HA, D], f32, tag="z1c", bufs=2, name=f"y1{g}{p}")
                    w1e = W16B(f"{p}W1e")
                    for jb in range(8):
                        nc.tensor.matmul(y1[:], w1e[:, jb * HA:(jb + 1) * HA],
                                         Mt[jb][:], start=(jb == 0), stop=False)
                    nc.tensor.matmul(y1[:], W16(f"plhw_{p[0]}", 2),
                                     W16(f"fm{g}", 2), start=False, stop=True)
                    tpl = sb.tile([HA, D], f16, tag=f"tpl{g}_{p}", name=f"tpl{g}_{p}")
                    nc.scalar.activation(tpl[:], y1[:], AF.Tanh,
                                         bias=W32(f"{p}b1", HA))
                    yp = ps.tile([1, D], f32, tag="z2c", bufs=3, name=f"yp{g}{p}")
                    nc.tensor.matmul(yp[:], W16(f"plW2_{p[0]}", HA), tpl[:],
                                     start=True, stop=True)
                    nc.scalar.activation(out_sb[g][:, osl], yp[:], AF.Identity,
                                         bias=W32(f"{p}b2", 1))

            # ---- GNN, chunk-pipelined, graphs interleaved ----
            def At_rhs(g, jb, lo, width):
                base = (jb // 4) * 4096 + (jb % 4) * N
                return At[g][:, base + lo: base + lo + width]

            h_nm = {g: None for g in range(BL)}
            for l in range(3):
                din = IN if l == 0 else H
                qb, t_fm, h_new = {}, {}, {}
                for g in range(BL):
                    qb[g] = sb.tile([din, N], f16, tag="qb", bufs=3,
                                    name=f"qb{g}_{l}")
                    t_fm[g] = sb.tile([128, N], f16, tag="t", bufs=3,
                                      name=f"t{g}_{l}")
                    h_new[g] = sb.tile([128, N], f16, tag="h", bufs=4,
                                       name=f"h{g}_{l}")
                for c in range(2):
                    for g in range(BL):
                        cs = slice(c * 512, (c + 1) * 512)
                        q = ps.tile([128, 512], f32, tag="qc", bufs=3,
                                    name=f"q{g}_{l}_{c}")
                        for jb in range(8):
                            if l == 0:
                                b0 = F16_COLS["h0"][0] + (g * 8 + jb) * IN
                                lhsT = wf16[:, b0:b0 + IN]
                            else:
                                lhsT = h_nm[g][:, jb * H:(jb + 1) * H]
                            nc.tensor.matmul(q[:din, :], lhsT,
                                             At_rhs(g, jb, c * 512, 512),
                                             start=(jb == 0), stop=(jb == 7))
                        nc.vector.tensor_copy(qb[g][:, cs], q[:din, :])
                        z1 = ps.tile([128, 512], f32, tag="z1c", bufs=2,
                                     name=f"z1{g}_{l}_{c}")
                        nc.tensor.matmul(z1[:], W16(f"g{l}W1", din),
                                         qb[g][:, cs], start=True, stop=True)
                        nc.scalar.activation(t_fm[g][:, cs], z1[:], AF.Relu,
                                             bias=W32(f"g{l}b1", H))
                        z2 = ps.tile([128, 512], f32, tag="z2c", bufs=3,
                                     name=f"z2{g}_{l}_{c}")
                        for k in range(4):
                            ib = 4 * c + k
                            nc.tensor.matmul(
                                z2[:, k * H:(k + 1) * H],
                                t_fm[g][:, ib * H:(ib + 1) * H],
                                W16(f"g{l}W2", H), start=True, stop=True)
                        ht = sb.tile([128, 512], f16, tag="ht", bufs=3,
                                     name=f"ht{g}_{l}_{c}")
                        nc.vector.tensor_add(ht[:], z2[:], b2rep[l][:, cs])
                        nc.scalar.activation(h_new[g][:, cs], ht[:], AF.Relu)
                for g in range(BL):
                    h_nm[g] = h_new[g]

            for g in range(BL):
                # ---- pool + candidate gather ----
                P = ps.tile([128, 1 + J], f32, tag="z1c", bufs=2, name=f"P{g}")
                for jb in range(8):
                    nc.tensor.matmul(P[:], h_nm[g][:, jb * H:(jb + 1) * H],
                                     Rt[g, jb][:], start=(jb == 0),
                                     stop=(jb == 7))
                Pb = sb.tile([128, 1 + J], f16, tag=f"Pb{g}")
                nc.vector.tensor_copy(Pb[:], P[:])

                # ---- actor head ----
                zcf = ps.tile([HA, J], f32, tag="z2c", bufs=3, name=f"zcf{g}")
                nc.tensor.matmul(zcf[:], W16("aW1t"), Pb[:, 1:1 + J],
                                 start=True, stop=True)
                zhp = ps.tile([HA, 1], f32, tag="qc", bufs=3, name=f"zhp{g}")
                nc.tensor.matmul(zhp[:], W16("aW1b"), Pb[:, 0:1],
                                 start=True, stop=True)
                bias_a = sb.tile([HA, 1], f32, tag=f"biasa{g}")
                nc.vector.tensor_add(bias_a[:], zhp[:], W32("ab1", HA))
                ta = sb.tile([HA, J], f16, tag=f"ta{g}")
                nc.scalar.activation(ta[:], zcf[:], AF.Tanh, bias=bias_a[:])
                scp = ps.tile([1, J], f32, tag="qc", bufs=3, name=f"scp{g}")
                nc.tensor.matmul(scp[:], W16("aW2", HA), ta[:],
                                 start=True, stop=True)
                nc.scalar.activation(out_sb[g][:, 0:J], scp[:], AF.Identity,
                                     bias=W32("ab2", 1))

                # ---- critic head ----
                zv = ps.tile([HC, 1], f32, tag="qc", bufs=3, name=f"zv{g}")
                nc.tensor.matmul(zv[:], W16("cW1"), Pb[:, 0:1],
                                 start=True, stop=True)
                tv = sb.tile([HC, 1], f16, tag=f"tv{g}")
                nc.scalar.activation(tv[:], zv[:], AF.Tanh, bias=W32("cb1", HC))
                vv = ps.tile([1, 1], f32, tag="qc", bufs=3, name=f"vv{g}")
                nc.tensor.matmul(vv[:], W16("cW2", HC), tv[:],
                                 start=True, stop=True)
                nc.scalar.activation(out_sb[g][:, J:J + 1], vv[:], AF.Identity,
                                     bias=W32("cb2", 1))
                nc.sync.dma_start(out_e[g:g + 1, :], out_sb[g][:])

    nc.compile()
    return nc


def _flatten_params(params):
    out = {}
    for l, layer in enumerate(params["gnn"]):
        for k, v in layer.items():
            out[f"gnn{l}_{k}"] = v
    for head in ("actor", "critic", "actorPL", "criticPL"):
        for k, v in params[head].items():
            out[f"{head}_{k}"] = v
    return out


def _prep_in_maps(state_ft, state_fm, candidate, mask, adj, graph_pool, params):
    f16 = np.float16
    state_ft = np.asarray(state_ft, np.float32)
    state_fm = np.asarray(state_fm, np.float32)
    candidate_np = np.asarray(candidate)
    adj = np.asarray(adj, np.float32)
    graph_pool = np.asarray(graph_pool, np.float32)
    P = {k: np.asarray(v, np.float32) for k, v in _flatten_params(params).items()}

    w16 = np.zeros((128, C16), f16)
    w32 = np.zeros((128, C32), np.float32)
    w16b = np.zeros((128, C16B), f16)
    b2b = np.zeros((3, N), np.float32)

    def put16(name, arr):
        c0, cn = F16_COLS[name]
        w16[:arr.shape[0], c0:c0 + arr.shape[1]] = arr.astype(f16)

    def put32(name, arr):
        c0, cn = F32_COLS[name]
        w32[:arr.shape[0], c0:c0 + arr.shape[1]] = arr

    for l in range(3):
        put16(f"g{l}W1", P[f"gnn{l}_W1"])
        put16(f"g{l}W2", P[f"gnn{l}_W2"])
        b2b[l, :] = np.tile(P[f"gnn{l}_b2"], 8)
        put32(f"g{l}b1", P[f"gnn{l}_b1"][:, None])
    put16("aW1t", P["actor_W1"][:H]); put16("aW1b", P["actor_W1"][H:])
    put16("aW2", P["actor_W2"]); put16("cW1", P["critic_W1"])
    put16("cW2", P["critic_W2"])
    put32("iota8", np.arange(8, dtype=np.float32)[None, :] * 128
          + np.arange(128, dtype=np.float32)[:, None])
    put32("ab1", P["actor_b1"][:, None]); put32("ab2", P["actor_b2"][:, None])
    put32("cb1", P["critic_b1"][:, None]); put32("cb2", P["critic_b2"][:, None])
    put32("iota16", np.broadcast_to(
        np.arange(D, dtype=np.float32), (128, D)).copy())
    for p, nm in (("apl", "actorPL"), ("cpl", "criticPL")):
        w1e = P[f"{nm}_W1"][2::2]                      # (N, HA)
        c0, cn = F16B_COLS[f"{p}W1e"]
        w16b[:, c0:c0 + cn] = np.ascontiguousarray(
            w1e.reshape(8, 128, HA).transpose(1, 0, 2)
            .reshape(128, 8 * HA)).astype(f16)
        put16(f"plhw_{p[0]}", P[f"{nm}_W1"][:2])
        put16(f"plW2_{p[0]}", P[f"{nm}_W2"])
        put32(f"{p}b1", P[f"{nm}_b1"][:, None])
        put32(f"{p}b2", P[f"{nm}_b2"][:, None])

    sft3 = state_ft.reshape(B, N, IN)
    fm3 = state_fm.reshape(B, D, 2)
    in_maps = []
    for c in range(NCORES):
        sl = slice(c * BL, (c + 1) * BL)
        wc16 = w16.copy()
        wc32 = w32.copy()
        # sft layout: col = (g*8 + b)*3 + ch;  [128, 48]
        s = np.ascontiguousarray(
            sft3[sl].reshape(BL * 8, 128, IN).transpose(1, 0, 2)
            .reshape(128, BL * 8 * IN))
        c0 = F32_COLS["sft"][0]
        wc32[:, c0:c0 + BL * 8 * IN] = s
        c0 = F16_COLS["h0"][0]
        wc16[:, c0:c0 + BL * 8 * IN] = s.astype(f16)
        for g in range(BL):
            gg = c * BL + g
            c0, _ = F16_COLS[f"cr{g}"]
            wc16[:, c0:c0 + J] = candidate_np[gg].astype(f16)[None, :]
            c0, _ = F16_COLS[f"gp{g}"]
            wc16[:, c0:c0 + 8] = graph_pool[gg].reshape(8, 128).T.astype(f16)
            c0, _ = F16_COLS[f"fm{g}"]
            wc16[:2, c0:c0 + D] = fm3[gg].T.astype(f16)
        # adjT pre-shuffled to the exact SBUF layout: row p holds, for each
        # half hf and block b, AT[512*hf + 128*b + p, :]  (AT = adj.T)
        a = adj[sl].transpose(0, 2, 1).reshape(BL, 2, 4, 128, N)
        a = a.transpose(0, 3, 1, 2, 4).reshape(BL, 128, 8192)
        in_maps.append({
            "wf16": wc16, "wf32": wc32, "wf16b": w16b, "b2b": b2b,
            "adjT": np.ascontiguousarray(a).astype(f16),
        })
    return in_maps


def _install_ntff_hook():
    import sys, types
    if "antenv.axon_hooks" in sys.modules:
        return
    try:
        import antenv
        mod = types.ModuleType("antenv.axon_hooks")
        mod._hook = None
        mod.set_axon_ntff_profile_hook = lambda h: setattr(mod, "_hook", h)
        mod.get_axon_ntff_profile_hook = lambda: mod._hook
        sys.modules["antenv.axon_hooks"] = mod
        antenv.axon_hooks = mod
        from trn_agent_boot.trn_boot import _ntff_profile_via_ctypes
        mod.set_axon_ntff_profile_hook(
            _ntff_profile_via_ctypes("/opt/axon/libaxon_pjrt.so"))
    except Exception:
        pass


def _gumbel_noise():
    if "gumbel" not in _cache:
        import jax
        k1, k2 = jax.random.split(jax.random.key(42))
        g1 = np.asarray(jax.random.gumbel(k1, (B, J), np.float32))
        g2 = np.asarray(jax.random.gumbel(k2, (B, D), np.float32))
        _cache["gumbel"] = (g1, g2)
    return _cache["gumbel"]


def _run_device(in_maps, trace=False):
    from concourse.bass_utils import run_bass_kernel_spmd
    if "nc" not in _cache:
        _cache["nc"] = _build_nc()
    if trace:
        _install_ntff_hook()
    res = run_bass_kernel_spmd(
        _cache["nc"], in_maps, core_ids=list(range(NCORES)), trace=trace)
    O = np.concatenate([r["outs"] for r in res.results], 0)   # (B, 161)
    S = O[:, 0:J]
    V = O[:, J:J + 1]
    DS = O[:, J + 1:J + 1 + D]
    CR = O[:, J + 1 + D:J + 1 + 2 * D]
    return (S, V, DS, CR), res


def _host_tail(S, V, DS, CR, candidate, mask):
    mask = np.asarray(mask, bool)
    cand = np.asarray(candidate)
    g1, g2 = _gumbel_noise()
    logits = np.where(mask, -np.inf, S)
    zmax = np.max(logits, axis=1, keepdims=True)
    e = np.exp(logits - zmax)
    e[~np.isfinite(logits)] = 0.0
    esum = e.sum(1, keepdims=True)
    pi = (e / esum)[..., None].astype(np.float32)
    task_ix = np.argmax(logits + g1, axis=1).astype(np.int32)
    lse = zmax[:, 0] + np.log(esum[:, 0])
    logp = (np.take_along_axis(logits, task_ix[:, None].astype(np.int64), 1)[:, 0]
            - lse).astype(np.float32)
    sel = np.take_along_axis(cand, task_ix[:, None].astype(np.int64), 1)[:, 0]
    sel = sel.astype(np.int32)
    dmax = DS.max(1, keepdims=True)
    ed = np.exp(DS - dmax)
    edsum = ed.sum(1, keepdims=True)
    mhi = (ed / edsum)[..., None].astype(np.float32)
    device_ID = np.argmax(DS + g2, axis=1).astype(np.int32)
    mh_logp = (np.take_along_axis(DS, device_ID[:, None].astype(np.int64), 1)[:, 0]
               - (dmax[:, 0] + np.log(edsum[:, 0]))).astype(np.float32)
    vm = CR.min(1).astype(np.float32)
    v = V.astype(np.float32).reshape(B, 1)
    return (sel, task_ix, pi, v, logp, device_ID, mhi, vm, mh_logp)


def kernel(state_ft, state_fm, candidate, mask, adj, graph_pool, params):
    in_maps = _prep_in_maps(state_ft, state_fm, candidate, mask, adj,
                            graph_pool, params)
    (S, V, DS, CR), _ = _run_device(in_maps)
    return _host_tail(S, V, DS, CR, candidate, mask)
